# revision 4
# baseline (speedup 1.0000x reference)
"""Multi-head self-attention (B=1, S=4096, D=2048, H=16, Dh=128) on 8 TRN2
NeuronCores. Head-sharded tensor parallelism: each core computes 2 heads end to
end in transposed layout, writes its partial out-projection [D, S]; the host
sums the 8 partials and transposes back to [S, D].

Dtype strategy: activations/weights stream as bf16 (matmul inputs), all matmul
accumulation is fp32 in PSUM; softmax statistics accumulate in bf16 on the DVE
(2x mode) and are partition-reduced on the GpSimd/Pool engine so the Activation
engine runs exp back-to-back.  Attention scores are computed in S^T layout
[k, q] so the softmax reduction needs no transposes anywhere.

Scheduling notes:
- One flat pool scope: no mid-kernel pool-drain barriers.  PSUM runs on four
  shared [128,1024] tags (pA/pB = scores double-buffer, pC/pD = PV accumulator
  + out-proj staging, alternating per (qb,h)).
- The (qb,h)-boundary normalize chain (exp -> bf16 adds -> gpsimd all-reduce
  ~6.7us -> reciprocal -> PV scale) is long; the next block's interleaved
  out-proj starts at kt=11 so the chain never blocks the in-order PE queue.
- Engine budget per core (phase 2): Scalar=exp only (~1.13us/kt), PE=scores+
  PV+out-proj (~1.07us/kt), DVE=bf16 adds+evictions, Pool=partition reductions.
"""
import sys
import numpy as np

for _p in ("/opt/trn_rl_repo",):
    if _p not in sys.path:
        sys.path.append(_p)

import concourse.bacc as bacc
import concourse.mybir as mybir
import concourse.tile as tile
from concourse import bass_isa

F32 = mybir.dt.float32
F32R = mybir.dt.float32r
BF16 = mybir.dt.bfloat16
AF = mybir.ActivationFunctionType
MUL = mybir.AluOpType.mult
RADD = bass_isa.ReduceOp.add

D = 2048            # d_model
S = 4096            # sequence length
DH = 128            # head dim
HPC = 2             # heads per core
DHC = HPC * DH      # 256 head-dims per core
NC = 8              # cores
EPS = 1e-6
SCALE = 1.0 / np.sqrt(DH)

NCH = S // 512      # 8 seq chunks of 512
KT_D = D // 128     # 16 k-tiles over d_model
KT_S = S // 128     # 32 k-tiles over sequence

# kt slots (within the following (qb,h) block) at which the previous q-block's
# out-projection tiles are emitted: late enough that the previous block's
# normalize chain has drained, early enough to spread the PE burst.
OP_KTS = {11 + 2 * j: j for j in range(8)}

TRACE = False       # set by test harness for profiling runs


def build():
    nc = bacc.Bacc("TRN2", target_bir_lowering=False, debug=False)

    xTb = nc.dram_tensor("xTb", [D, S], BF16, kind="ExternalInput")
    wqb = nc.dram_tensor("wqb", [D, DHC], BF16, kind="ExternalInput")
    wkb = nc.dram_tensor("wkb", [D, DHC], BF16, kind="ExternalInput")
    wvb = nc.dram_tensor("wvb", [D, DHC], BF16, kind="ExternalInput")
    wob = nc.dram_tensor("wob", [DHC, D], BF16, kind="ExternalInput")
    qw = nc.dram_tensor("qw", [DH, 1], F32, kind="ExternalInput")
    kw = nc.dram_tensor("kw", [DH, 1], F32, kind="ExternalInput")
    ones_c_d = nc.dram_tensor("ones_c", [128, 1], BF16, kind="ExternalInput")
    outT = nc.dram_tensor("outT", [D, S], F32, kind="ExternalOutput")

    # batched-DMA views: whole weight matrices / x chunks in one transfer
    xTb_c = xTb.rearrange("(kt p) s -> p kt s", p=128)      # [128,16,4096]
    wq_a = wqb.rearrange("(kt p) m -> p kt m", p=128)       # [128,16,256]
    wk_a = wkb.rearrange("(kt p) m -> p kt m", p=128)
    wv_a = wvb.rearrange("(kt p) m -> p kt m", p=128)
    wo_a = wob.rearrange("(kt p) m -> p kt m", p=128)       # [128,2,2048]
    outT_t = outT.rearrange("(mo p) s -> mo p s", p=128)    # [16,128,4096]

    with tile.TileContext(nc) as tc, \
         nc.allow_low_precision(reason="bf16 compute is intentional"):
        with (
            tc.tile_pool(name="consts", bufs=1) as consts,
            tc.tile_pool(name="big", bufs=1) as big,
            tc.tile_pool(name="stream", bufs=6) as stream,
            tc.tile_pool(name="ev", bufs=1) as ev,
            tc.tile_pool(name="ps", bufs=1, space="PSUM") as ps,
        ):
            # ---- residents ----
            ones_col = consts.tile([128, 1], BF16)         # lhsT for rms sums
            eps_sb = consts.tile([1, 1], F32, tag="eps")
            qw_sb = consts.tile([DH, 1], F32, tag="qw")    # per-partition norm w
            kw_sb = consts.tile([DH, 1], F32, tag="kw")

            qT = [big.tile([128, S], BF16, tag=f"q{h}", name=f"qT{h}")
                  for h in range(HPC)]
            kT = [big.tile([128, S], BF16, tag=f"k{h}", name=f"kT{h}")
                  for h in range(HPC)]
            v_sb = big.tile([128, KT_S, DHC], BF16, tag="v")
            o_sb = [big.tile([128, S], BF16, tag=f"o{h}", name=f"o{h}")
                    for h in range(HPC)]
            wo_sb = big.tile([128, HPC, D], BF16, tag="wo")
            wq_sb = big.tile([128, KT_D, DHC], BF16, tag="wq")
            wk_sb = big.tile([128, KT_D, DHC], BF16, tag="wk")
            wv_sb = big.tile([128, KT_D, DHC], BF16, tag="wv")

            # ========== Phase 1: q/k/v projections + q/k rmsnorm ==========
            # Single pass over x^T: per (chunk, kt) one x tile feeds 2 q-mms,
            # 2 k-mms and 4 v-mms.  The 4 v accumulators pack two [128,256]
            # groups per PSUM bank.  Issue order: first chunk's x right after
            # wq/wk/wv so the first matmul can start ~12us in.
            nc.sync.dma_start(out=wq_sb[:], in_=wq_a)
            nc.sync.dma_start(out=wk_sb[:], in_=wk_a)
            nc.sync.dma_start(out=wv_sb[:], in_=wv_a)
            xc = [None] * NCH
            xc[0] = stream.tile([128, KT_D, 512], BF16, tag="xc", bufs=2,
                                name="xc0")
            nc.sync.dma_start(out=xc[0][:], in_=xTb_c[:, :, 0:512])
            nc.sync.dma_start(out=qw_sb[:], in_=qw[:])
            nc.sync.dma_start(out=kw_sb[:], in_=kw[:])
            nc.sync.dma_start(out=ones_col[:], in_=ones_c_d[:])
            nc.vector.memset(eps_sb[:], EPS)
            nc.sync.dma_start(out=wo_sb[:], in_=wo_a)

            for n in range(NCH):
                sl = slice(n * 512, (n + 1) * 512)
                if n + 1 < NCH:
                    xc[n + 1] = stream.tile([128, KT_D, 512], BF16,
                                            tag="xc", bufs=2,
                                            name=f"xc{n+1}")
                    nc.sync.dma_start(
                        out=xc[n + 1][:],
                        in_=xTb_c[:, :, (n + 1) * 512:(n + 2) * 512])
                kk = ps.tile([128, 1024], F32, tag="pA", name="kk")
                qq = ps.tile([128, 1024], F32, tag="pB", name="qq")
                vv = ps.tile([128, 1024], F32, tag="pC", name="vv")
                ps_k = [kk[:, m * 512:(m + 1) * 512] for m in range(HPC)]
                ps_q = [qq[:, m * 512:(m + 1) * 512] for m in range(HPC)]
                for kt in range(KT_D):
                    x_t = xc[n][:, kt, :]
                    for m in range(HPC):
                        ms = slice(m * DH, (m + 1) * DH)
                        nc.tensor.matmul(ps_k[m], wk_sb[:, kt, ms], x_t,
                                         start=(kt == 0), stop=(kt == KT_D - 1),
                                         skip_group_check=True)
                        nc.tensor.matmul(ps_q[m], wq_sb[:, kt, ms], x_t,
                                         start=(kt == 0), stop=(kt == KT_D - 1),
                                         skip_group_check=True)
                    for sm in range(4):
                        pv = vv[:, sm * 256:(sm + 1) * 256]
                        nc.tensor.matmul(pv,
                                         xc[n][:, kt, sm * 128:(sm + 1) * 128],
                                         wv_sb[:, kt, :],
                                         start=(kt == 0 and sm % 2 == 0),
                                         stop=(kt == KT_D - 1),
                                         skip_group_check=True)
                # rmsnorm + evict k first (phase 2's first scores need kT),
                # then q; stt reads the PSUM accumulators directly
                for ps_list, dst, w_col in ((ps_k, kT, kw_sb), (ps_q, qT, qw_sb)):
                    for m in range(HPC):
                        sq = ev.tile([128, 512], BF16, tag="sq", bufs=2)
                        nc.scalar.activation(sq[:], ps_list[m], AF.Square)
                        ps_ss = ps.tile([1, 512], F32, tag="pD", name="ps_ss")
                        nc.tensor.matmul(ps_ss[:], ones_col[:], sq[:],
                                         start=True, stop=True,
                                         skip_group_check=True)
                        ms_row = ev.tile([1, 512], F32, tag="msr", bufs=2)
                        nc.scalar.activation(ms_row[:], ps_ss[:], AF.Identity,
                                             bias=eps_sb[:], scale=1.0 / 128.0)
                        rec = ev.tile([1, 512], F32, tag="rec", bufs=2)
                        nc.vector.reciprocal_approx_fast(out=rec[:], in_=ms_row[:])
                        rrms = ev.tile([1, 512], F32R, tag="rrms", bufs=2)
                        nc.scalar.activation(rrms[:], rec[:], AF.Sqrt)
                        rb = ev.tile([128, 512], F32R, tag="rb", bufs=2)
                        nc.gpsimd.partition_broadcast(rb[:], rrms[:])
                        nc.vector.scalar_tensor_tensor(
                            dst[m][:, sl], ps_list[m], w_col[:], rb[:],
                            op0=MUL, op1=MUL)
                # evict v on scalar (idle in phase 1)
                for i in range(2):
                    nc.scalar.copy(
                        v_sb[:, n * 4 + 2 * i:n * 4 + 2 * i + 2, :].rearrange(
                            "p a b -> p (a b)"),
                        vv[:, i * 512:(i + 1) * 512])

            # ============ Phase 2+3: attention + out-projection ============
            # 1024-wide q blocks; S^T scores span two PSUM banks. Scalar does
            # ONLY exp; softmax sums accumulate in bf16 on DVE (2x mode) and
            # partition-reduce on GpSimd.
            NQB = S // 1024

            def outproj_mo(qb, mo, ytag, yeng):
                mosl = slice(mo * 128, (mo + 1) * 128)
                ps_y = ps.tile([128, 1024], F32, tag=ytag, name="ps_y")
                for h2 in range(HPC):
                    for u in range(2):
                        usl = slice(qb * 1024 + u * 512,
                                    qb * 1024 + (u + 1) * 512)
                        nc.tensor.matmul(ps_y[:, u * 512:(u + 1) * 512],
                                         wo_sb[:, h2, mosl], o_sb[h2][:, usl],
                                         start=(h2 == 0), stop=(h2 == HPC - 1),
                                         skip_group_check=True)
                y = stream.tile([128, 1024], F32, tag="y", bufs=4)
                if yeng == "s":
                    nc.scalar.copy(y[:], ps_y[:])
                else:
                    nc.vector.tensor_copy(y[:], ps_y[:])
                nc.sync.dma_start(out=outT_t[mo][:, qb * 1024:(qb + 1) * 1024],
                                  in_=y[:])

            for qb in range(NQB):
                qsl = slice(qb * 1024, (qb + 1) * 1024)
                for h in range(HPC):
                    idx = qb * HPC + h
                    pso_tag = "pC" if idx % 2 == 0 else "pD"
                    oth_tag = "pD" if idx % 2 == 0 else "pC"
                    ps_o = ps.tile([128, 1024], F32, tag=pso_tag,
                                   name=f"ps_o{idx % 2}")
                    acc = ev.tile([128, 1024], BF16, tag="acc", bufs=2,
                                  name="acc")
                    pt_prev = None

                    def emit_pv(kt2, pt2):
                        for u in range(2):
                            nc.tensor.matmul(ps_o[:, u * 512:(u + 1) * 512],
                                             v_sb[:, kt2, h * DH:(h + 1) * DH],
                                             pt2[:, u * 512:(u + 1) * 512],
                                             start=(kt2 == 0),
                                             stop=(kt2 == KT_S - 1),
                                             skip_group_check=True)

                    pv_pend = []
                    for kt in range(KT_S):
                        ksl = slice(kt * 128, (kt + 1) * 128)
                        ps_s = ps.tile([128, 1024], F32,
                                       tag=("pA" if kt % 2 == 0 else "pB"),
                                       name="ps_s")
                        for u in range(2):
                            usl = slice(qb * 1024 + u * 512,
                                        qb * 1024 + (u + 1) * 512)
                            nc.tensor.matmul(ps_s[:, u * 512:(u + 1) * 512],
                                             kT[h][:, ksl], qT[h][:, usl],
                                             start=True, stop=True,
                                             skip_group_check=True)
                        pt = stream.tile([128, 1024], BF16, tag="pt", bufs=8)
                        nc.scalar.activation(pt[:], ps_s[:], AF.Exp, scale=SCALE)
                        if kt % 2 == 0:
                            pt_prev = pt
                        else:
                            pair = ev.tile([128, 1024], BF16, tag="pair",
                                           bufs=2, name="pair")
                            nc.vector.tensor_add(pair[:], pt_prev[:], pt[:])
                            if kt == 1:
                                nc.vector.tensor_copy(acc[:], pair[:])
                            else:
                                nc.vector.tensor_add(acc[:], acc[:], pair[:])
                        pv_pend.append((kt, pt))
                        if len(pv_pend) > 2:
                            emit_pv(*pv_pend.pop(0))
                        if qb > 0 and kt in OP_KTS:
                            outproj_mo(qb - 1, h * 8 + OP_KTS[kt],
                                       oth_tag, "v")
                    for kt2, pt2 in pv_pend:
                        emit_pv(kt2, pt2)
                    # softmax denominator: bf16 acc -> partition all-reduce
                    # (GpSimd) -> reciprocal (DVE) -> scale PV (DVE)
                    se_b = ev.tile([128, 1024], F32, tag="seb", bufs=2,
                                   name="se_b")
                    nc.gpsimd.partition_all_reduce(se_b[:], acc[:],
                                                   channels=128,
                                                   reduce_op=RADD)
                    rb2 = ev.tile([128, 1024], F32, tag="rb2", bufs=2)
                    nc.vector.reciprocal_approx_fast(out=rb2[:], in_=se_b[:])
                    nc.vector.tensor_mul(o_sb[h][:, qsl], ps_o[:], rb2[:])

            # tail: last q block's out-projection, double-buffered across
            # the pC/pD tags with evictions split over vector+scalar
            for mo in range(D // 128):
                outproj_mo(NQB - 1, mo, "pC" if mo % 2 else "pD",
                           "s" if mo % 2 else "v")

    nc.compile()
    return nc


_NC_CACHE = None


def _get_nc():
    global _NC_CACHE
    if _NC_CACHE is None:
        _NC_CACHE = build()
    return _NC_CACHE


def _ensure_axon_hooks_stub():
    """bass_utils imports antenv.axon_hooks when tracing is requested via env;
    provide a no-op stub if the image lacks it so a stray BASS_TRACE cannot
    crash the run."""
    import types
    try:
        from antenv import axon_hooks  # noqa: F401
        return
    except Exception:
        pass
    try:
        import antenv
        m = types.ModuleType("antenv.axon_hooks")
        m.set_axon_ntff_profile_hook = lambda h: None
        m.get_axon_ntff_profile_hook = lambda: None
        sys.modules["antenv.axon_hooks"] = m
        antenv.axon_hooks = m
    except Exception:
        pass


def kernel(x, wq, wk, wv, wo, q_norm_w, k_norm_w):
    import ml_dtypes
    from concourse import bass_utils

    _ensure_axon_hooks_stub()

    x = np.asarray(x, dtype=np.float32)
    wq = np.asarray(wq, dtype=np.float32)
    wk = np.asarray(wk, dtype=np.float32)
    wv = np.asarray(wv, dtype=np.float32)
    wo = np.asarray(wo, dtype=np.float32)
    q_norm_w = np.asarray(q_norm_w, dtype=np.float32).reshape(DH, 1)
    k_norm_w = np.asarray(k_norm_w, dtype=np.float32).reshape(DH, 1)

    B = x.shape[0]
    xTb = np.ascontiguousarray(x.reshape(S, D).T).astype(ml_dtypes.bfloat16)

    in_maps = []
    for c in range(NC):
        hsl = slice(c * DHC, (c + 1) * DHC)
        in_maps.append({
            "xTb": xTb,
            "wqb": np.ascontiguousarray(wq[hsl, :].T).astype(ml_dtypes.bfloat16),
            "wkb": np.ascontiguousarray(wk[hsl, :].T).astype(ml_dtypes.bfloat16),
            "wvb": np.ascontiguousarray(wv[hsl, :].T).astype(ml_dtypes.bfloat16),
            "wob": np.ascontiguousarray(wo[:, hsl].T).astype(ml_dtypes.bfloat16),
            "qw": q_norm_w,
            "kw": k_norm_w,
            "ones_c": np.ones((128, 1), dtype=ml_dtypes.bfloat16),
        })

    nc = _get_nc()
    res = bass_utils.run_bass_kernel_spmd(
        nc, in_maps, core_ids=list(range(NC)), trace=TRACE,
    )
    acc = res.results[0]["outT"]
    for c in range(1, NC):
        acc = acc + res.results[c]["outT"]
    out = np.ascontiguousarray(acc.T).reshape(B, S, D)
    if TRACE:
        kernel.last_exec_time_ns = res.exec_time_ns
        kernel.last_results = res
    return out


# revision 7
# speedup vs baseline: 1.0307x; 1.0307x over previous
"""Multi-head self-attention (B=1, S=4096, D=2048, H=16, Dh=128) on 8 TRN2
NeuronCores. Head-sharded tensor parallelism: each core computes 2 heads end to
end in transposed layout, writes its partial out-projection [D, S]; the host
sums the 8 partials and transposes back to [S, D].

Dtype strategy: activations/weights stream as bf16 (matmul inputs), all matmul
accumulation is fp32 in PSUM; softmax statistics accumulate in bf16 on the DVE
(2x mode) and are partition-reduced on the GpSimd/Pool engine so the Activation
engine runs exp back-to-back.  Attention scores are computed in S^T layout
[k, q] so the softmax reduction needs no transposes anywhere.

Scheduling notes:
- One flat pool scope: no mid-kernel pool-drain barriers.  PSUM runs on four
  shared [128,1024] tags (pA/pB = scores double-buffer, pC/pD = PV accumulator
  + out-proj staging, alternating per (qb,h)).
- q/k/v activations live in PER-CHUNK tiles so phase-2 reads depend only on
  the producing chunk's eviction, not on the whole phase-1 sweep.
- The (qb,h) normalize chain is long (gpsimd all-reduce ~6.7us); its reciprocal
  + PV-scale are DEFERRED into the next block's kt loop (kt=6) so they never
  head-of-line-block the in-order DVE queue, and the next block's interleaved
  out-proj starts at kt=11, after the previous normalize has drained.
- Engine budget per core (phase 2): Scalar=exp only (~1.13us/kt), PE=scores+
  PV+out-proj (~1.07us/kt), DVE=bf16 adds+evictions, Pool=partition reductions.
"""
import sys
import numpy as np

for _p in ("/opt/trn_rl_repo",):
    if _p not in sys.path:
        sys.path.append(_p)

import concourse.bacc as bacc
import concourse.mybir as mybir
import concourse.tile as tile
from concourse import bass_isa

F32 = mybir.dt.float32
F32R = mybir.dt.float32r
BF16 = mybir.dt.bfloat16
AF = mybir.ActivationFunctionType
MUL = mybir.AluOpType.mult
RADD = bass_isa.ReduceOp.add

D = 2048            # d_model
S = 4096            # sequence length
DH = 128            # head dim
HPC = 2             # heads per core
DHC = HPC * DH      # 256 head-dims per core
NC = 8              # cores
EPS = 1e-6
SCALE = 1.0 / np.sqrt(DH)

NCH = S // 512      # 8 seq chunks of 512
KT_D = D // 128     # 16 k-tiles over d_model
KT_S = S // 128     # 32 k-tiles over sequence

# kt slots (within the following (qb,h) block) at which the previous q-block's
# out-projection tiles are emitted: late enough that the previous block's
# normalize chain has drained, early enough to spread the PE burst.
OP_KTS = {11 + 2 * j: j for j in range(8)}
NORM_KT = 6         # kt at which the previous block's recip+scale are emitted

TRACE = False       # set by test harness for profiling runs


def build():
    nc = bacc.Bacc("TRN2", target_bir_lowering=False, debug=False)

    xTb = nc.dram_tensor("xTb", [D, S], BF16, kind="ExternalInput")
    wqb = nc.dram_tensor("wqb", [D, DHC], BF16, kind="ExternalInput")
    wkb = nc.dram_tensor("wkb", [D, DHC], BF16, kind="ExternalInput")
    wvb = nc.dram_tensor("wvb", [D, DHC], BF16, kind="ExternalInput")
    wob = nc.dram_tensor("wob", [DHC, D], BF16, kind="ExternalInput")
    qw = nc.dram_tensor("qw", [DH, 1], F32, kind="ExternalInput")
    kw = nc.dram_tensor("kw", [DH, 1], F32, kind="ExternalInput")
    ones_c_d = nc.dram_tensor("ones_c", [128, 1], BF16, kind="ExternalInput")
    outT = nc.dram_tensor("outT", [D, S], F32, kind="ExternalOutput")

    # batched-DMA views: whole weight matrices / x chunks in one transfer
    xTb_c = xTb.rearrange("(kt p) s -> p kt s", p=128)      # [128,16,4096]
    wq_a = wqb.rearrange("(kt p) m -> p kt m", p=128)       # [128,16,256]
    wk_a = wkb.rearrange("(kt p) m -> p kt m", p=128)
    wv_a = wvb.rearrange("(kt p) m -> p kt m", p=128)
    wo_a = wob.rearrange("(kt p) m -> p kt m", p=128)       # [128,2,2048]
    outT_t = outT.rearrange("(mo p) s -> mo p s", p=128)    # [16,128,4096]

    with tile.TileContext(nc) as tc, \
         nc.allow_low_precision(reason="bf16 compute is intentional"):
        with (
            tc.tile_pool(name="consts", bufs=1) as consts,
            tc.tile_pool(name="big", bufs=1) as big,
            tc.tile_pool(name="stream", bufs=6) as stream,
            tc.tile_pool(name="ev", bufs=1) as ev,
            tc.tile_pool(name="ps", bufs=1, space="PSUM") as ps,
        ):
            # ---- residents ----
            ones_col = consts.tile([128, 1], BF16)         # lhsT for rms sums
            eps_sb = consts.tile([1, 1], F32, tag="eps")
            qw_sb = consts.tile([DH, 1], F32, tag="qw")    # per-partition norm w
            kw_sb = consts.tile([DH, 1], F32, tag="kw")

            # per-chunk activation tiles: fine-grained phase-2 dependencies
            qTc = [[big.tile([128, 512], BF16, tag=f"q{h}c{c}", name=f"qT{h}_{c}")
                    for c in range(NCH)] for h in range(HPC)]
            kTc = [[big.tile([128, 512], BF16, tag=f"k{h}c{c}", name=f"kT{h}_{c}")
                    for c in range(NCH)] for h in range(HPC)]
            vc = [big.tile([128, 4, DHC], BF16, tag=f"vc{c}", name=f"v_{c}")
                  for c in range(NCH)]
            o_sb = [big.tile([128, S], BF16, tag=f"o{h}", name=f"o{h}")
                    for h in range(HPC)]
            wo_sb = big.tile([128, HPC, D], BF16, tag="wo")
            wq_sb = big.tile([128, KT_D, DHC], BF16, tag="wq")
            wk_sb = big.tile([128, KT_D, DHC], BF16, tag="wk")
            wv_sb = big.tile([128, KT_D, DHC], BF16, tag="wv")

            # ========== Phase 1: q/k/v projections + q/k rmsnorm ==========
            # Single pass over x^T: per (chunk, kt) one x tile feeds 2 q-mms,
            # 2 k-mms and 4 v-mms.  The 4 v accumulators pack two [128,256]
            # groups per PSUM bank.  DMAs execute serially on the queue, so
            # issue order is wq, x0, wk, wv (first matmul needs wq+x0 only).
            nc.sync.dma_start(out=wq_sb[:], in_=wq_a)
            xc = [None] * NCH
            xc[0] = stream.tile([128, KT_D, 512], BF16, tag="xc", bufs=2,
                                name="xc0")
            nc.sync.dma_start(out=xc[0][:], in_=xTb_c[:, :, 0:512])
            nc.sync.dma_start(out=wk_sb[:], in_=wk_a)
            nc.sync.dma_start(out=wv_sb[:], in_=wv_a)
            nc.sync.dma_start(out=qw_sb[:], in_=qw[:])
            nc.sync.dma_start(out=kw_sb[:], in_=kw[:])
            nc.sync.dma_start(out=ones_col[:], in_=ones_c_d[:])
            nc.vector.memset(eps_sb[:], EPS)
            nc.sync.dma_start(out=wo_sb[:], in_=wo_a)

            for n in range(NCH):
                if n + 1 < NCH:
                    xc[n + 1] = stream.tile([128, KT_D, 512], BF16,
                                            tag="xc", bufs=2,
                                            name=f"xc{n+1}")
                    nc.sync.dma_start(
                        out=xc[n + 1][:],
                        in_=xTb_c[:, :, (n + 1) * 512:(n + 2) * 512])
                kk = ps.tile([128, 1024], F32, tag="pA", name="kk")
                qq = ps.tile([128, 1024], F32, tag="pB", name="qq")
                vv = ps.tile([128, 1024], F32, tag="pC", name="vv")
                ps_k = [kk[:, m * 512:(m + 1) * 512] for m in range(HPC)]
                ps_q = [qq[:, m * 512:(m + 1) * 512] for m in range(HPC)]
                for kt in range(KT_D):
                    x_t = xc[n][:, kt, :]
                    for m in range(HPC):
                        ms = slice(m * DH, (m + 1) * DH)
                        nc.tensor.matmul(ps_k[m], wk_sb[:, kt, ms], x_t,
                                         start=(kt == 0), stop=(kt == KT_D - 1),
                                         skip_group_check=True)
                        nc.tensor.matmul(ps_q[m], wq_sb[:, kt, ms], x_t,
                                         start=(kt == 0), stop=(kt == KT_D - 1),
                                         skip_group_check=True)
                    for sm in range(4):
                        pv = vv[:, sm * 256:(sm + 1) * 256]
                        nc.tensor.matmul(pv,
                                         xc[n][:, kt, sm * 128:(sm + 1) * 128],
                                         wv_sb[:, kt, :],
                                         start=(kt == 0 and sm % 2 == 0),
                                         stop=(kt == KT_D - 1),
                                         skip_group_check=True)
                # rmsnorm + evict k first (phase 2's first scores need kT).
                # Square (scalar) + raw copy (DVE) release the PSUM quickly;
                # the long stats chain runs on the SBUF copy off-path.
                for ps_list, dst, w_col in ((ps_k, kTc, kw_sb), (ps_q, qTc, qw_sb)):
                    for m in range(HPC):
                        sq = ev.tile([128, 512], BF16, tag="sq", bufs=2)
                        nc.scalar.activation(sq[:], ps_list[m], AF.Square)
                        raw = ev.tile([128, 512], F32, tag="raw", bufs=2)
                        nc.vector.tensor_copy(raw[:], ps_list[m])
                        ps_ss = ps.tile([1, 512], F32, tag="pD", name="ps_ss")
                        nc.tensor.matmul(ps_ss[:], ones_col[:], sq[:],
                                         start=True, stop=True,
                                         skip_group_check=True)
                        ms_row = ev.tile([1, 512], F32, tag="msr", bufs=2)
                        nc.scalar.activation(ms_row[:], ps_ss[:], AF.Identity,
                                             bias=eps_sb[:], scale=1.0 / 128.0)
                        rec = ev.tile([1, 512], F32, tag="rec", bufs=2)
                        nc.vector.reciprocal_approx_fast(out=rec[:], in_=ms_row[:])
                        rrms = ev.tile([1, 512], F32R, tag="rrms", bufs=2)
                        nc.scalar.activation(rrms[:], rec[:], AF.Sqrt)
                        rb = ev.tile([128, 512], F32R, tag="rb", bufs=2)
                        nc.gpsimd.partition_broadcast(rb[:], rrms[:])
                        nc.vector.scalar_tensor_tensor(
                            dst[m][n][:], raw[:], w_col[:], rb[:],
                            op0=MUL, op1=MUL)
                # evict v on scalar (idle in phase 1)
                for i in range(2):
                    nc.scalar.copy(
                        vc[n][:, 2 * i:2 * i + 2, :].rearrange("p a b -> p (a b)"),
                        vv[:, i * 512:(i + 1) * 512])

            # ============ Phase 2+3: attention + out-projection ============
            # 1024-wide q blocks; S^T scores span two PSUM banks. Scalar does
            # ONLY exp; softmax sums accumulate in bf16 on DVE (2x mode) and
            # partition-reduce on GpSimd.
            NQB = S // 1024

            def outproj_mo(qb, mo, ytag, yeng):
                mosl = slice(mo * 128, (mo + 1) * 128)
                ps_y = ps.tile([128, 1024], F32, tag=ytag, name="ps_y")
                for h2 in range(HPC):
                    for u in range(2):
                        nc.tensor.matmul(ps_y[:, u * 512:(u + 1) * 512],
                                         wo_sb[:, h2, mosl],
                                         o_sb[h2][:, qb * 1024 + u * 512:
                                                   qb * 1024 + (u + 1) * 512],
                                         start=(h2 == 0), stop=(h2 == HPC - 1),
                                         skip_group_check=True)
                y = stream.tile([128, 1024], F32, tag="y", bufs=4)
                if yeng == "s":
                    nc.scalar.copy(y[:], ps_y[:])
                else:
                    nc.vector.tensor_copy(y[:], ps_y[:])
                nc.sync.dma_start(out=outT_t[mo][:, qb * 1024:(qb + 1) * 1024],
                                  in_=y[:])

            pending = None      # (ps_o, se_b, h, qsl) of the previous block

            def flush_norm(p):
                ps_o_p, se_b_p, h_p, qsl_p = p
                rb2 = ev.tile([128, 1024], F32, tag="rb2", bufs=1)
                nc.vector.reciprocal_approx_fast(out=rb2[:], in_=se_b_p[:])
                nc.vector.tensor_mul(o_sb[h_p][:, qsl_p], ps_o_p[:], rb2[:])

            for qb in range(NQB):
                qsl = slice(qb * 1024, (qb + 1) * 1024)
                for h in range(HPC):
                    idx = qb * HPC + h
                    pso_tag = "pC" if idx % 2 == 0 else "pD"
                    oth_tag = "pD" if idx % 2 == 0 else "pC"
                    ps_o = ps.tile([128, 1024], F32, tag=pso_tag,
                                   name=f"ps_o{idx % 2}")
                    acc = ev.tile([128, 1024], BF16, tag="acc", bufs=2,
                                  name="acc")
                    pt_prev = None

                    def emit_pv(kt2, pt2):
                        for u in range(2):
                            nc.tensor.matmul(ps_o[:, u * 512:(u + 1) * 512],
                                             vc[kt2 // 4][:, kt2 % 4,
                                                          h * DH:(h + 1) * DH],
                                             pt2[:, u * 512:(u + 1) * 512],
                                             start=(kt2 == 0),
                                             stop=(kt2 == KT_S - 1),
                                             skip_group_check=True)

                    pv_pend = []
                    for kt in range(KT_S):
                        kq = (kt % 4) * 128
                        ps_s = ps.tile([128, 1024], F32,
                                       tag=("pA" if kt % 2 == 0 else "pB"),
                                       name="ps_s")
                        for u in range(2):
                            nc.tensor.matmul(ps_s[:, u * 512:(u + 1) * 512],
                                             kTc[h][kt // 4][:, kq:kq + 128],
                                             qTc[h][qb * 2 + u][:],
                                             start=True, stop=True,
                                             skip_group_check=True)
                        pt = stream.tile([128, 1024], BF16, tag="pt", bufs=8)
                        nc.scalar.activation(pt[:], ps_s[:], AF.Exp, scale=SCALE)
                        if kt % 2 == 0:
                            pt_prev = pt
                        else:
                            pair = ev.tile([128, 1024], BF16, tag="pair",
                                           bufs=2, name="pair")
                            nc.vector.tensor_add(pair[:], pt_prev[:], pt[:])
                            if kt == 1:
                                nc.vector.tensor_copy(acc[:], pair[:])
                            else:
                                nc.vector.tensor_add(acc[:], acc[:], pair[:])
                        pv_pend.append((kt, pt))
                        if len(pv_pend) > 2:
                            emit_pv(*pv_pend.pop(0))
                        if kt == NORM_KT and pending is not None:
                            flush_norm(pending)
                            pending = None
                        if qb > 0 and kt in OP_KTS:
                            outproj_mo(qb - 1, h * 8 + OP_KTS[kt],
                                       oth_tag, "v")
                    for kt2, pt2 in pv_pend:
                        emit_pv(kt2, pt2)
                    # softmax denominator: bf16 acc -> partition all-reduce
                    # (GpSimd); reciprocal + PV scale deferred to next block
                    se_b = ev.tile([128, 1024], F32, tag="seb", bufs=2,
                                   name="se_b")
                    nc.gpsimd.partition_all_reduce(se_b[:], acc[:],
                                                   channels=128,
                                                   reduce_op=RADD)
                    pending = (ps_o, se_b, h, qsl)

            flush_norm(pending)
            # tail: last q block's out-projection, double-buffered across
            # the pC/pD tags with evictions split over vector+scalar
            for mo in range(D // 128):
                outproj_mo(NQB - 1, mo, "pC" if mo % 2 else "pD",
                           "s" if mo % 2 else "v")

    nc.compile()
    return nc


_NC_CACHE = None


def _get_nc():
    global _NC_CACHE
    if _NC_CACHE is None:
        _NC_CACHE = build()
    return _NC_CACHE


def _ensure_axon_hooks_stub():
    """bass_utils imports antenv.axon_hooks when tracing is requested via env;
    provide a no-op stub if the image lacks it so a stray BASS_TRACE cannot
    crash the run."""
    import types
    try:
        from antenv import axon_hooks  # noqa: F401
        return
    except Exception:
        pass
    try:
        import antenv
        m = types.ModuleType("antenv.axon_hooks")
        m.set_axon_ntff_profile_hook = lambda h: None
        m.get_axon_ntff_profile_hook = lambda: None
        sys.modules["antenv.axon_hooks"] = m
        antenv.axon_hooks = m
    except Exception:
        pass


def kernel(x, wq, wk, wv, wo, q_norm_w, k_norm_w):
    import ml_dtypes
    from concourse import bass_utils

    _ensure_axon_hooks_stub()

    x = np.asarray(x, dtype=np.float32)
    wq = np.asarray(wq, dtype=np.float32)
    wk = np.asarray(wk, dtype=np.float32)
    wv = np.asarray(wv, dtype=np.float32)
    wo = np.asarray(wo, dtype=np.float32)
    q_norm_w = np.asarray(q_norm_w, dtype=np.float32).reshape(DH, 1)
    k_norm_w = np.asarray(k_norm_w, dtype=np.float32).reshape(DH, 1)

    B = x.shape[0]
    xTb = np.ascontiguousarray(x.reshape(S, D).T).astype(ml_dtypes.bfloat16)

    in_maps = []
    for c in range(NC):
        hsl = slice(c * DHC, (c + 1) * DHC)
        in_maps.append({
            "xTb": xTb,
            "wqb": np.ascontiguousarray(wq[hsl, :].T).astype(ml_dtypes.bfloat16),
            "wkb": np.ascontiguousarray(wk[hsl, :].T).astype(ml_dtypes.bfloat16),
            "wvb": np.ascontiguousarray(wv[hsl, :].T).astype(ml_dtypes.bfloat16),
            "wob": np.ascontiguousarray(wo[:, hsl].T).astype(ml_dtypes.bfloat16),
            "qw": q_norm_w,
            "kw": k_norm_w,
            "ones_c": np.ones((128, 1), dtype=ml_dtypes.bfloat16),
        })

    nc = _get_nc()
    res = bass_utils.run_bass_kernel_spmd(
        nc, in_maps, core_ids=list(range(NC)), trace=TRACE,
    )
    acc = res.results[0]["outT"]
    for c in range(1, NC):
        acc = acc + res.results[c]["outT"]
    out = np.ascontiguousarray(acc.T).reshape(B, S, D)
    if TRACE:
        kernel.last_exec_time_ns = res.exec_time_ns
        kernel.last_results = res
    return out


# revision 11
# speedup vs baseline: 1.0499x; 1.0186x over previous
"""Multi-head self-attention (B=1, S=4096, D=2048, H=16, Dh=128) on 8 TRN2
NeuronCores. Head-sharded tensor parallelism: each core computes 2 heads end to
end in transposed layout, writes its partial out-projection [D, S]; the host
sums the 8 partials and transposes back to [S, D].

Dtype strategy: activations/weights stream as bf16 (matmul inputs), all matmul
accumulation is fp32 in PSUM; softmax statistics accumulate in bf16 on the DVE
(2x mode) and are partition-reduced on the GpSimd/Pool engine so the Activation
engine runs exp back-to-back.  Attention scores are computed in S^T layout
[k, q] so the softmax reduction needs no transposes anywhere.

Scheduling notes:
- One flat pool scope: no mid-kernel pool-drain barriers.  PSUM runs on four
  shared [128,1024] tags (pA/pB = scores double-buffer, pC/pD = PV accumulator
  + out-proj staging, alternating per (qb,h)).
- q/k/v activations live in PER-CHUNK tiles so phase-2 reads depend only on
  the producing chunk's eviction, not on the whole phase-1 sweep.
- The (qb,h) normalize chain is long (gpsimd all-reduce ~6.7us); its reciprocal
  + PV-scale are DEFERRED into the next block's kt loop (kt=6) so they never
  head-of-line-block the in-order DVE queue, and the next block's interleaved
  out-proj starts at kt=11, after the previous normalize has drained.
- Engine budget per core (phase 2): Scalar=exp only (~1.13us/kt), PE=scores+
  PV+out-proj (~1.07us/kt), DVE=bf16 adds+evictions, Pool=partition reductions.
"""
import sys
import numpy as np

for _p in ("/opt/trn_rl_repo",):
    if _p not in sys.path:
        sys.path.append(_p)

import concourse.bacc as bacc
import concourse.mybir as mybir
import concourse.tile as tile
from concourse import bass_isa

F32 = mybir.dt.float32
F32R = mybir.dt.float32r
BF16 = mybir.dt.bfloat16
AF = mybir.ActivationFunctionType
MUL = mybir.AluOpType.mult
RADD = bass_isa.ReduceOp.add

D = 2048            # d_model
S = 4096            # sequence length
DH = 128            # head dim
HPC = 2             # heads per core
DHC = HPC * DH      # 256 head-dims per core
NC = 8              # cores
EPS = 1e-6
SCALE = 1.0 / np.sqrt(DH)

NCH = S // 512      # 8 seq chunks of 512
KT_D = D // 128     # 16 k-tiles over d_model
KT_S = S // 128     # 32 k-tiles over sequence

# kt slots (within the following (qb,h) block) at which the previous q-block's
# out-projection tiles are emitted: late enough that the previous block's
# normalize chain has drained, early enough to spread the PE burst.
OP_KTS = {11 + 2 * j: j for j in range(8)}
NORM_KT = 6         # kt at which the previous block's recip+scale are emitted

TRACE = False       # set by test harness for profiling runs


def build():
    nc = bacc.Bacc("TRN2", target_bir_lowering=False, debug=False)

    xTb = nc.dram_tensor("xTb", [D, S], BF16, kind="ExternalInput")
    wqb = nc.dram_tensor("wqb", [D, DHC], BF16, kind="ExternalInput")
    wkb = nc.dram_tensor("wkb", [D, DHC], BF16, kind="ExternalInput")
    wvb = nc.dram_tensor("wvb", [D, DHC], BF16, kind="ExternalInput")
    wob = nc.dram_tensor("wob", [DHC, D], BF16, kind="ExternalInput")
    qw = nc.dram_tensor("qw", [DH, 1], F32, kind="ExternalInput")
    kw = nc.dram_tensor("kw", [DH, 1], F32, kind="ExternalInput")
    ones_c_d = nc.dram_tensor("ones_c", [128, 1], BF16, kind="ExternalInput")
    outT = nc.dram_tensor("outT", [D, S], F32, kind="ExternalOutput")

    # batched-DMA views: whole weight matrices / x chunks in one transfer
    xTb_c = xTb.rearrange("(kt p) s -> p kt s", p=128)      # [128,16,4096]
    wq_a = wqb.rearrange("(kt p) m -> p kt m", p=128)       # [128,16,256]
    wk_a = wkb.rearrange("(kt p) m -> p kt m", p=128)
    wv_a = wvb.rearrange("(kt p) m -> p kt m", p=128)
    wo_a = wob.rearrange("(kt p) m -> p kt m", p=128)       # [128,2,2048]
    outT_t = outT.rearrange("(mo p) s -> mo p s", p=128)    # [16,128,4096]

    with tile.TileContext(nc) as tc, \
         nc.allow_low_precision(reason="bf16 compute is intentional"):
        with (
            tc.tile_pool(name="consts", bufs=1) as consts,
            tc.tile_pool(name="big", bufs=1) as big,
            tc.tile_pool(name="stream", bufs=6) as stream,
            tc.tile_pool(name="ev", bufs=1) as ev,
            tc.tile_pool(name="ps", bufs=1, space="PSUM") as ps,
        ):
            # ---- residents ----
            ones_col = consts.tile([128, 1], BF16)         # lhsT for rms sums
            eps_sb = consts.tile([1, 1], F32, tag="eps")
            qw_sb = consts.tile([DH, 1], F32, tag="qw")    # per-partition norm w
            kw_sb = consts.tile([DH, 1], F32, tag="kw")

            # per-chunk activation tiles: fine-grained phase-2 dependencies
            qTc = [[big.tile([128, 512], BF16, tag=f"q{h}c{c}", name=f"qT{h}_{c}")
                    for c in range(NCH)] for h in range(HPC)]
            kTc = [[big.tile([128, 512], BF16, tag=f"k{h}c{c}", name=f"kT{h}_{c}")
                    for c in range(NCH)] for h in range(HPC)]
            vc = [big.tile([128, 4, DHC], BF16, tag=f"vc{c}", name=f"v_{c}")
                  for c in range(NCH)]
            o_sb = [big.tile([128, S], BF16, tag=f"o{h}", name=f"o{h}")
                    for h in range(HPC)]
            wo_sb = big.tile([128, HPC, D], BF16, tag="wo")
            wq_sb = big.tile([128, KT_D, DHC], BF16, tag="wq")
            wk_sb = big.tile([128, KT_D, DHC], BF16, tag="wk")
            wv_sb = big.tile([128, KT_D, DHC], BF16, tag="wv")

            # ========== Phase 1: q/k/v projections + q/k rmsnorm ==========
            # Single pass over x^T: per (chunk, kt) one x tile feeds 2 q-mms,
            # 2 k-mms and 4 v-mms.  The 4 v accumulators pack two [128,256]
            # groups per PSUM bank.  DMAs execute serially on the queue, so
            # issue order is wq, x0, wk, wv (first matmul needs wq+x0 only).
            nc.sync.dma_start(out=wq_sb[:], in_=wq_a)
            xc = [None] * NCH
            xc[0] = stream.tile([128, KT_D, 512], BF16, tag="xc", bufs=2,
                                name="xc0")
            nc.sync.dma_start(out=xc[0][:], in_=xTb_c[:, :, 0:512])
            nc.sync.dma_start(out=wk_sb[:], in_=wk_a)
            nc.sync.dma_start(out=wv_sb[:], in_=wv_a)
            nc.sync.dma_start(out=qw_sb[:], in_=qw[:])
            nc.sync.dma_start(out=kw_sb[:], in_=kw[:])
            nc.sync.dma_start(out=ones_col[:], in_=ones_c_d[:])
            nc.vector.memset(eps_sb[:], EPS)
            nc.sync.dma_start(out=wo_sb[:], in_=wo_a)

            for n in range(NCH):
                if n + 1 < NCH:
                    xc[n + 1] = stream.tile([128, KT_D, 512], BF16,
                                            tag="xc", bufs=2,
                                            name=f"xc{n+1}")
                    nc.sync.dma_start(
                        out=xc[n + 1][:],
                        in_=xTb_c[:, :, (n + 1) * 512:(n + 2) * 512])
                kk = ps.tile([128, 1024], F32, tag="pA", name="kk")
                qq = ps.tile([128, 1024], F32, tag="pB", name="qq")
                vv = ps.tile([128, 1024], F32, tag="pC", name="vv")
                ps_k = [kk[:, m * 512:(m + 1) * 512] for m in range(HPC)]
                ps_q = [qq[:, m * 512:(m + 1) * 512] for m in range(HPC)]
                for kt in range(KT_D):
                    x_t = xc[n][:, kt, :]
                    for m in range(HPC):
                        ms = slice(m * DH, (m + 1) * DH)
                        nc.tensor.matmul(ps_k[m], wk_sb[:, kt, ms], x_t,
                                         start=(kt == 0), stop=(kt == KT_D - 1),
                                         skip_group_check=True)
                        nc.tensor.matmul(ps_q[m], wq_sb[:, kt, ms], x_t,
                                         start=(kt == 0), stop=(kt == KT_D - 1),
                                         skip_group_check=True)
                    for sm in range(4):
                        pv = vv[:, sm * 256:(sm + 1) * 256]
                        nc.tensor.matmul(pv,
                                         xc[n][:, kt, sm * 128:(sm + 1) * 128],
                                         wv_sb[:, kt, :],
                                         start=(kt == 0 and sm % 2 == 0),
                                         stop=(kt == KT_D - 1),
                                         skip_group_check=True)
                # rmsnorm + evict k first (phase 2's first scores need kT).
                # Square (scalar) + raw copy (DVE) release the PSUM quickly;
                # the long stats chain runs on the SBUF copy off-path.
                for ps_list, dst, w_col in ((ps_k, kTc, kw_sb), (ps_q, qTc, qw_sb)):
                    for m in range(HPC):
                        sq = ev.tile([128, 512], BF16, tag="sq", bufs=2)
                        nc.scalar.activation(sq[:], ps_list[m], AF.Square)
                        raw = ev.tile([128, 512], F32, tag="raw", bufs=2)
                        nc.vector.tensor_copy(raw[:], ps_list[m])
                        ps_ss = ps.tile([1, 512], F32, tag="pD", name="ps_ss")
                        nc.tensor.matmul(ps_ss[:], ones_col[:], sq[:],
                                         start=True, stop=True,
                                         skip_group_check=True)
                        ms_row = ev.tile([1, 512], F32, tag="msr", bufs=2)
                        nc.scalar.activation(ms_row[:], ps_ss[:], AF.Identity,
                                             bias=eps_sb[:], scale=1.0 / 128.0)
                        rec = ev.tile([1, 512], F32, tag="rec", bufs=2)
                        nc.vector.reciprocal_approx_fast(out=rec[:], in_=ms_row[:])
                        rrms = ev.tile([1, 512], F32R, tag="rrms", bufs=2)
                        nc.scalar.activation(rrms[:], rec[:], AF.Sqrt)
                        rb = ev.tile([128, 512], F32R, tag="rb", bufs=2)
                        nc.gpsimd.partition_broadcast(rb[:], rrms[:])
                        nc.vector.scalar_tensor_tensor(
                            dst[m][n][:], raw[:], w_col[:], rb[:],
                            op0=MUL, op1=MUL)
                # evict v on scalar (idle in phase 1)
                for i in range(2):
                    nc.scalar.copy(
                        vc[n][:, 2 * i:2 * i + 2, :].rearrange("p a b -> p (a b)"),
                        vv[:, i * 512:(i + 1) * 512])

            # ============ Phase 2+3: attention + out-projection ============
            # 1024-wide q blocks; S^T scores span two PSUM banks. Scalar does
            # ONLY exp; softmax sums accumulate in bf16 on DVE (2x mode) and
            # partition-reduce on GpSimd.
            NQB = S // 1024

            def outproj_mo(qb, mo, ytag, yeng):
                mosl = slice(mo * 128, (mo + 1) * 128)
                ps_y = ps.tile([128, 1024], F32, tag=ytag, name="ps_y")
                for h2 in range(HPC):
                    for u in range(2):
                        nc.tensor.matmul(ps_y[:, u * 512:(u + 1) * 512],
                                         wo_sb[:, h2, mosl],
                                         o_sb[h2][:, qb * 1024 + u * 512:
                                                   qb * 1024 + (u + 1) * 512],
                                         start=(h2 == 0), stop=(h2 == HPC - 1),
                                         skip_group_check=True)
                y = stream.tile([128, 1024], F32, tag="y", bufs=4)
                if yeng == "s":
                    nc.scalar.copy(y[:], ps_y[:])
                else:
                    nc.vector.tensor_copy(y[:], ps_y[:])
                nc.sync.dma_start(out=outT_t[mo][:, qb * 1024:(qb + 1) * 1024],
                                  in_=y[:])

            for qb in range(NQB):
                qsl = slice(qb * 1024, (qb + 1) * 1024)
                for h in range(HPC):
                    idx = qb * HPC + h
                    pso_tag = "pC" if idx % 2 == 0 else "pD"
                    oth_tag = "pD" if idx % 2 == 0 else "pC"
                    ps_o = ps.tile([128, 1024], F32, tag=pso_tag,
                                   name=f"ps_o{idx % 2}")
                    acc = ev.tile([128, 1024], BF16, tag="acc", bufs=2,
                                  name="acc")
                    pt_prev = None

                    def emit_pv(kt2, pt2):
                        for u in range(2):
                            nc.tensor.matmul(ps_o[:, u * 512:(u + 1) * 512],
                                             vc[kt2 // 4][:, kt2 % 4,
                                                          h * DH:(h + 1) * DH],
                                             pt2[:, u * 512:(u + 1) * 512],
                                             start=(kt2 == 0),
                                             stop=(kt2 == KT_S - 1),
                                             skip_group_check=True)

                    pv_pend = []
                    for kt in range(KT_S):
                        kq = (kt % 4) * 128
                        ps_s = ps.tile([128, 1024], F32,
                                       tag=("pA" if kt % 2 == 0 else "pB"),
                                       name="ps_s")
                        for u in range(2):
                            nc.tensor.matmul(ps_s[:, u * 512:(u + 1) * 512],
                                             kTc[h][kt // 4][:, kq:kq + 128],
                                             qTc[h][qb * 2 + u][:],
                                             start=True, stop=True,
                                             skip_group_check=True)
                        pt = stream.tile([128, 1024], BF16, tag="pt", bufs=8)
                        nc.scalar.activation(pt[:], ps_s[:], AF.Exp, scale=SCALE)
                        if kt % 2 == 0:
                            pt_prev = pt
                        else:
                            pair = ev.tile([128, 1024], BF16, tag="pair",
                                           bufs=2, name="pair")
                            nc.vector.tensor_add(pair[:], pt_prev[:], pt[:])
                            if kt == 1:
                                nc.vector.tensor_copy(acc[:], pair[:])
                            else:
                                nc.vector.tensor_add(acc[:], acc[:], pair[:])
                        pv_pend.append((kt, pt))
                        if len(pv_pend) > 2:
                            emit_pv(*pv_pend.pop(0))
                        if qb > 0 and kt in OP_KTS:
                            outproj_mo(qb - 1, h * 8 + OP_KTS[kt],
                                       oth_tag, "v")
                    for kt2, pt2 in pv_pend:
                        emit_pv(kt2, pt2)
                    # softmax denominator: ones-matmul partition sum (PE, into
                    # the pB score slot; the next block's odd-kt scores only
                    # wait the fast row copy) -> row reciprocal (DVE) ->
                    # broadcast (GpSimd) -> PV scale (DVE)
                    ps_se = ps.tile([1, 1024], F32, tag="pB", name="ps_se")
                    for u in range(2):
                        nc.tensor.matmul(ps_se[:, u * 512:(u + 1) * 512],
                                         ones_col[:],
                                         acc[:, u * 512:(u + 1) * 512],
                                         start=True, stop=True,
                                         skip_group_check=True)
                    se_row = ev.tile([1, 1024], F32, tag="ser", bufs=1)
                    nc.vector.tensor_copy(se_row[:], ps_se[:])
                    rec2 = ev.tile([1, 1024], F32, tag="rec2", bufs=1)
                    nc.vector.reciprocal_approx_fast(out=rec2[:], in_=se_row[:])
                    rb2 = ev.tile([128, 1024], F32, tag="rb2", bufs=2)
                    nc.gpsimd.partition_broadcast(rb2[:], rec2[:])
                    nc.vector.tensor_mul(o_sb[h][:, qsl], ps_o[:], rb2[:])

            # tail: last q block's out-projection, double-buffered across
            # the pC/pD tags with evictions split over vector+scalar
            for mo in range(D // 128):
                outproj_mo(NQB - 1, mo, "pC" if mo % 2 else "pD",
                           "s" if mo % 2 else "v")

    nc.compile()
    return nc


_NC_CACHE = None


def _get_nc():
    global _NC_CACHE
    if _NC_CACHE is None:
        _NC_CACHE = build()
    return _NC_CACHE


def _ensure_axon_hooks_stub():
    """bass_utils imports antenv.axon_hooks when tracing is requested via env;
    provide a no-op stub if the image lacks it so a stray BASS_TRACE cannot
    crash the run."""
    import types
    try:
        from antenv import axon_hooks  # noqa: F401
        return
    except Exception:
        pass
    try:
        import antenv
        m = types.ModuleType("antenv.axon_hooks")
        m.set_axon_ntff_profile_hook = lambda h: None
        m.get_axon_ntff_profile_hook = lambda: None
        sys.modules["antenv.axon_hooks"] = m
        antenv.axon_hooks = m
    except Exception:
        pass


def kernel(x, wq, wk, wv, wo, q_norm_w, k_norm_w):
    import ml_dtypes
    from concourse import bass_utils

    _ensure_axon_hooks_stub()

    x = np.asarray(x, dtype=np.float32)
    wq = np.asarray(wq, dtype=np.float32)
    wk = np.asarray(wk, dtype=np.float32)
    wv = np.asarray(wv, dtype=np.float32)
    wo = np.asarray(wo, dtype=np.float32)
    q_norm_w = np.asarray(q_norm_w, dtype=np.float32).reshape(DH, 1)
    k_norm_w = np.asarray(k_norm_w, dtype=np.float32).reshape(DH, 1)

    B = x.shape[0]
    xTb = np.ascontiguousarray(x.reshape(S, D).T).astype(ml_dtypes.bfloat16)

    in_maps = []
    for c in range(NC):
        hsl = slice(c * DHC, (c + 1) * DHC)
        in_maps.append({
            "xTb": xTb,
            "wqb": np.ascontiguousarray(wq[hsl, :].T).astype(ml_dtypes.bfloat16),
            "wkb": np.ascontiguousarray(wk[hsl, :].T).astype(ml_dtypes.bfloat16),
            "wvb": np.ascontiguousarray(wv[hsl, :].T).astype(ml_dtypes.bfloat16),
            "wob": np.ascontiguousarray(wo[:, hsl].T).astype(ml_dtypes.bfloat16),
            "qw": q_norm_w,
            "kw": k_norm_w,
            "ones_c": np.ones((128, 1), dtype=ml_dtypes.bfloat16),
        })

    nc = _get_nc()
    res = bass_utils.run_bass_kernel_spmd(
        nc, in_maps, core_ids=list(range(NC)), trace=TRACE,
    )
    acc = res.results[0]["outT"]
    for c in range(1, NC):
        acc = acc + res.results[c]["outT"]
    out = np.ascontiguousarray(acc.T).reshape(B, S, D)
    if TRACE:
        kernel.last_exec_time_ns = res.exec_time_ns
        kernel.last_results = res
    return out


# revision 15
# speedup vs baseline: 1.1068x; 1.0542x over previous
"""Multi-head self-attention (B=1, S=4096, D=2048, H=16, Dh=128) on 8 TRN2
NeuronCores. Head-sharded tensor parallelism: each core computes 2 heads end to
end in transposed layout, writes its partial out-projection [D, S]; the host
sums the 8 partials and transposes back to [S, D].

Dtype strategy: activations/weights stream as bf16 (matmul inputs), all matmul
accumulation is fp32 in PSUM; softmax statistics accumulate in bf16 on the DVE
(2x mode) and are partition-reduced on the GpSimd/Pool engine so the Activation
engine runs exp back-to-back.  Attention scores are computed in S^T layout
[k, q] so the softmax reduction needs no transposes anywhere.

Scheduling notes:
- One flat pool scope: no mid-kernel pool-drain barriers.  PSUM runs on four
  shared [128,1024] tags (pA/pB = scores double-buffer, pC/pD = PV accumulator
  + out-proj staging, alternating per (qb,h)).
- q/k/v activations live in PER-CHUNK tiles so phase-2 reads depend only on
  the producing chunk's eviction, not on the whole phase-1 sweep.
- The (qb,h) normalize chain is long (gpsimd all-reduce ~6.7us); its reciprocal
  + PV-scale are DEFERRED into the next block's kt loop (kt=6) so they never
  head-of-line-block the in-order DVE queue, and the next block's interleaved
  out-proj starts at kt=11, after the previous normalize has drained.
- Engine budget per core (phase 2): Scalar=exp only (~1.13us/kt), PE=scores+
  PV+out-proj (~1.07us/kt), DVE=bf16 adds+evictions, Pool=partition reductions.
"""
import sys
import numpy as np

for _p in ("/opt/trn_rl_repo",):
    if _p not in sys.path:
        sys.path.append(_p)

import concourse.bacc as bacc
import concourse.mybir as mybir
import concourse.tile as tile
from concourse import bass_isa

F32 = mybir.dt.float32
F32R = mybir.dt.float32r
BF16 = mybir.dt.bfloat16
AF = mybir.ActivationFunctionType
MUL = mybir.AluOpType.mult
RADD = bass_isa.ReduceOp.add

D = 2048            # d_model
S = 4096            # sequence length
DH = 128            # head dim
HPC = 2             # heads per core
DHC = HPC * DH      # 256 head-dims per core
NC = 8              # cores
EPS = 1e-6
SCALE = 1.0 / np.sqrt(DH)

NCH = S // 512      # 8 seq chunks of 512
KT_D = D // 128     # 16 k-tiles over d_model
KT_S = S // 128     # 32 k-tiles over sequence

# kt slots (within the following (qb,h) block) at which the previous q-block's
# out-projection tiles are emitted: late enough that the previous block's
# normalize chain has drained, spread every 3rd kt to balance the PE burst
# against the exp cadence.
OP_KTS = {9 + 3 * j: j for j in range(8)}

TRACE = False       # set by test harness for profiling runs


def build():
    nc = bacc.Bacc("TRN2", target_bir_lowering=False, debug=False)

    xTb = nc.dram_tensor("xTb", [D, S], BF16, kind="ExternalInput")
    wqb = nc.dram_tensor("wqb", [D, DHC], BF16, kind="ExternalInput")
    wkb = nc.dram_tensor("wkb", [D, DHC], BF16, kind="ExternalInput")
    wvb = nc.dram_tensor("wvb", [D, DHC], BF16, kind="ExternalInput")
    wob = nc.dram_tensor("wob", [DHC, D], BF16, kind="ExternalInput")
    qw = nc.dram_tensor("qw", [DH, 1], F32, kind="ExternalInput")
    kw = nc.dram_tensor("kw", [DH, 1], F32, kind="ExternalInput")
    ones_c_d = nc.dram_tensor("ones_c", [128, 1], BF16, kind="ExternalInput")
    outT = nc.dram_tensor("outT", [D, S], F32, kind="ExternalOutput")

    # batched-DMA views: whole weight matrices / x chunks in one transfer
    xTb_c = xTb.rearrange("(kt p) s -> p kt s", p=128)      # [128,16,4096]
    wq_a = wqb.rearrange("(kt p) m -> p kt m", p=128)       # [128,16,256]
    wk_a = wkb.rearrange("(kt p) m -> p kt m", p=128)
    wv_a = wvb.rearrange("(kt p) m -> p kt m", p=128)
    wo_a = wob.rearrange("(kt p) m -> p kt m", p=128)       # [128,2,2048]
    outT_t = outT.rearrange("(mo p) s -> mo p s", p=128)    # [16,128,4096]

    with tile.TileContext(nc) as tc, \
         nc.allow_low_precision(reason="bf16 compute is intentional"):
        with (
            tc.tile_pool(name="consts", bufs=1) as consts,
            tc.tile_pool(name="big", bufs=1) as big,
            tc.tile_pool(name="stream", bufs=6) as stream,
            tc.tile_pool(name="ev", bufs=1) as ev,
            tc.tile_pool(name="ps", bufs=1, space="PSUM") as ps,
        ):
            # ---- residents ----
            ones_col = consts.tile([128, 1], BF16)         # lhsT for rms sums
            eps_sb = consts.tile([1, 1], F32, tag="eps")
            qw_sb = consts.tile([DH, 1], F32, tag="qw")    # per-partition norm w
            kw_sb = consts.tile([DH, 1], F32, tag="kw")

            # per-chunk activation tiles: fine-grained phase-2 dependencies
            qTc = [[big.tile([128, 512], BF16, tag=f"q{h}c{c}", name=f"qT{h}_{c}")
                    for c in range(NCH)] for h in range(HPC)]
            kTc = [[big.tile([128, 512], BF16, tag=f"k{h}c{c}", name=f"kT{h}_{c}")
                    for c in range(NCH)] for h in range(HPC)]
            vc = [big.tile([128, 4, DHC], BF16, tag=f"vc{c}", name=f"v_{c}")
                  for c in range(NCH)]
            o_sb = [big.tile([128, S], BF16, tag=f"o{h}", name=f"o{h}")
                    for h in range(HPC)]
            wo_sb = big.tile([128, HPC, D], BF16, tag="wo")
            wq_sb = big.tile([128, KT_D, DHC], BF16, tag="wq")
            wk_sb = big.tile([128, KT_D, DHC], BF16, tag="wk")
            wv_sb = big.tile([128, KT_D, DHC], BF16, tag="wv")

            # ========== Phase 1: q/k/v projections + q/k rmsnorm ==========
            # Single pass over x^T: per (chunk, kt) one x tile feeds 2 q-mms,
            # 2 k-mms and 4 v-mms.  The 4 v accumulators pack two [128,256]
            # groups per PSUM bank.  DMAs execute serially on the queue, so
            # issue order is wq, x0, wk, wv (first matmul needs wq+x0 only).
            nc.sync.dma_start(out=wq_sb[:], in_=wq_a)
            xc = [None] * NCH
            xc[0] = stream.tile([128, KT_D, 512], BF16, tag="xc", bufs=2,
                                name="xc0")
            nc.sync.dma_start(out=xc[0][:], in_=xTb_c[:, :, 0:512])
            nc.sync.dma_start(out=wk_sb[:], in_=wk_a)
            nc.sync.dma_start(out=wv_sb[:], in_=wv_a)
            nc.sync.dma_start(out=qw_sb[:], in_=qw[:])
            nc.sync.dma_start(out=kw_sb[:], in_=kw[:])
            nc.sync.dma_start(out=ones_col[:], in_=ones_c_d[:])
            nc.vector.memset(eps_sb[:], EPS)
            nc.sync.dma_start(out=wo_sb[:], in_=wo_a)

            for n in range(NCH):
                if n + 1 < NCH:
                    xc[n + 1] = stream.tile([128, KT_D, 512], BF16,
                                            tag="xc", bufs=2,
                                            name=f"xc{n+1}")
                    nc.sync.dma_start(
                        out=xc[n + 1][:],
                        in_=xTb_c[:, :, (n + 1) * 512:(n + 2) * 512])
                # k accumulator alternates pA/pD per chunk: the next chunk's
                # k-pass starts with no WAR wait on this chunk's evictions
                kk = ps.tile([128, 1024], F32, tag=("pA" if n % 2 == 0 else "pD"),
                             name="kk")
                qq = ps.tile([128, 1024], F32, tag="pB", name="qq")
                vv = ps.tile([128, 1024], F32, tag="pC", name="vv")
                ps_k = [kk[:, m * 512:(m + 1) * 512] for m in range(HPC)]
                ps_q = [qq[:, m * 512:(m + 1) * 512] for m in range(HPC)]
                # three passes over the resident x chunk: k, q, v
                for kt in range(KT_D):
                    for m in range(HPC):
                        nc.tensor.matmul(ps_k[m],
                                         wk_sb[:, kt, m * DH:(m + 1) * DH],
                                         xc[n][:, kt, :],
                                         start=(kt == 0), stop=(kt == KT_D - 1),
                                         skip_group_check=True)
                for kt in range(KT_D):
                    for m in range(HPC):
                        nc.tensor.matmul(ps_q[m],
                                         wq_sb[:, kt, m * DH:(m + 1) * DH],
                                         xc[n][:, kt, :],
                                         start=(kt == 0), stop=(kt == KT_D - 1),
                                         skip_group_check=True)
                for kt in range(KT_D):
                    for sm in range(4):
                        pv = vv[:, sm * 256:(sm + 1) * 256]
                        nc.tensor.matmul(pv,
                                         xc[n][:, kt, sm * 128:(sm + 1) * 128],
                                         wv_sb[:, kt, :],
                                         start=(kt == 0 and sm % 2 == 0),
                                         stop=(kt == KT_D - 1),
                                         skip_group_check=True)
                # rmsnorm stats + evictions.  Square (scalar) + raw copy (DVE)
                # release the PSUM accumulators quickly; the long stats chain
                # runs on the SBUF copies off-path.  The ss row sums live in
                # the pB rotation, emitted only after all qq accesses.
                sqs, raws = {}, {}
                for key, ps_list in (("k", ps_k), ("q", ps_q)):
                    for m in range(HPC):
                        sq = ev.tile([128, 512], BF16, tag="sq", bufs=4,
                                     name="sq")
                        nc.scalar.activation(sq[:], ps_list[m], AF.Square)
                        raw = ev.tile([128, 512], F32, tag="raw", bufs=4,
                                      name="raw")
                        nc.vector.tensor_copy(raw[:], ps_list[m])
                        sqs[key, m] = sq
                        raws[key, m] = raw
                for key, dst, w_col in (("k", kTc, kw_sb), ("q", qTc, qw_sb)):
                    for m in range(HPC):
                        ps_ss = ps.tile([1, 512], F32, tag="pB", name="ps_ss")
                        nc.tensor.matmul(ps_ss[:], ones_col[:], sqs[key, m][:],
                                         start=True, stop=True,
                                         skip_group_check=True)
                        ms_row = ev.tile([1, 512], F32, tag="msr", bufs=1)
                        nc.scalar.activation(ms_row[:], ps_ss[:], AF.Identity,
                                             bias=eps_sb[:], scale=1.0 / 128.0)
                        rec = ev.tile([1, 512], F32, tag="rec", bufs=1)
                        nc.vector.reciprocal_approx_fast(out=rec[:], in_=ms_row[:])
                        rrms = ev.tile([1, 512], F32R, tag="rrms", bufs=1)
                        nc.scalar.activation(rrms[:], rec[:], AF.Sqrt)
                        rb = ev.tile([128, 512], F32R, tag="rb", bufs=2)
                        nc.gpsimd.partition_broadcast(rb[:], rrms[:])
                        nc.vector.scalar_tensor_tensor(
                            dst[m][n][:], raws[key, m][:], w_col[:], rb[:],
                            op0=MUL, op1=MUL)
                # evict v on scalar (idle in phase 1)
                for i in range(2):
                    nc.scalar.copy(
                        vc[n][:, 2 * i:2 * i + 2, :].rearrange("p a b -> p (a b)"),
                        vv[:, i * 512:(i + 1) * 512])

            # ============ Phase 2+3: attention + out-projection ============
            # 1024-wide q blocks; S^T scores span two PSUM banks. Scalar does
            # ONLY exp; softmax sums accumulate in bf16 on DVE (2x mode) and
            # partition-reduce on GpSimd.
            NQB = S // 1024

            def outproj_mo(qb, mo, ytag, yeng):
                mosl = slice(mo * 128, (mo + 1) * 128)
                ps_y = ps.tile([128, 1024], F32, tag=ytag, name="ps_y")
                for h2 in range(HPC):
                    for u in range(2):
                        nc.tensor.matmul(ps_y[:, u * 512:(u + 1) * 512],
                                         wo_sb[:, h2, mosl],
                                         o_sb[h2][:, qb * 1024 + u * 512:
                                                   qb * 1024 + (u + 1) * 512],
                                         start=(h2 == 0), stop=(h2 == HPC - 1),
                                         skip_group_check=True)
                y = stream.tile([128, 1024], F32, tag="y", bufs=4)
                if yeng == "s":
                    nc.scalar.copy(y[:], ps_y[:])
                else:
                    nc.vector.tensor_copy(y[:], ps_y[:])
                nc.sync.dma_start(out=outT_t[mo][:, qb * 1024:(qb + 1) * 1024],
                                  in_=y[:])

            # Deferred finish of a (qb,h) block, interleaved into the next
            # block's early kt slots so the in-order engines never stall on
            # the cross-engine normalize chain.
            def make_finish(emit_pv_fn, pv_rest, acc_c, ps_o_c, h_c, qsl_c):
                st = {}

                def step(phase):
                    if phase == 0:          # leftover PV accumulations
                        for kt2, pt2 in pv_rest:
                            emit_pv_fn(kt2, pt2)
                    elif phase == 1:        # partition sum + reciprocal
                        ps_se = ps.tile([1, 1024], F32, tag="pB", name="ps_se")
                        for u in range(2):
                            nc.tensor.matmul(ps_se[:, u * 512:(u + 1) * 512],
                                             ones_col[:],
                                             acc_c[:, u * 512:(u + 1) * 512],
                                             start=True, stop=True,
                                             skip_group_check=True)
                        se_row = ev.tile([1, 1024], F32, tag="ser", bufs=1)
                        nc.vector.tensor_copy(se_row[:], ps_se[:])
                        rec2 = ev.tile([1, 1024], F32, tag="rec2", bufs=1)
                        nc.vector.reciprocal_approx_fast(out=rec2[:],
                                                         in_=se_row[:])
                        st["rec2"] = rec2
                    else:                   # broadcast + PV scale
                        rb2 = ev.tile([128, 1024], F32, tag="rb2", bufs=2)
                        nc.gpsimd.partition_broadcast(rb2[:], st["rec2"][:])
                        nc.vector.tensor_mul(o_sb[h_c][:, qsl_c],
                                             ps_o_c[:], rb2[:])
                return step

            finish = None
            for qb in range(NQB):
                qsl = slice(qb * 1024, (qb + 1) * 1024)
                for h in range(HPC):
                    idx = qb * HPC + h
                    pso_tag = "pC" if idx % 2 == 0 else "pD"
                    oth_tag = "pD" if idx % 2 == 0 else "pC"
                    ps_o = ps.tile([128, 1024], F32, tag=pso_tag,
                                   name=f"ps_o{idx % 2}")
                    acc = ev.tile([128, 1024], BF16, tag="acc", bufs=2,
                                  name="acc")
                    pt_prev = None

                    def emit_pv(kt2, pt2, ps_o=ps_o, h=h):
                        for u in range(2):
                            nc.tensor.matmul(ps_o[:, u * 512:(u + 1) * 512],
                                             vc[kt2 // 4][:, kt2 % 4,
                                                          h * DH:(h + 1) * DH],
                                             pt2[:, u * 512:(u + 1) * 512],
                                             start=(kt2 == 0),
                                             stop=(kt2 == KT_S - 1),
                                             skip_group_check=True)

                    pv_pend = []
                    for kt in range(KT_S):
                        kq = (kt % 4) * 128
                        ps_s = ps.tile([128, 1024], F32,
                                       tag=("pA" if kt % 2 == 0 else "pB"),
                                       name="ps_s")
                        for u in range(2):
                            nc.tensor.matmul(ps_s[:, u * 512:(u + 1) * 512],
                                             kTc[h][kt // 4][:, kq:kq + 128],
                                             qTc[h][qb * 2 + u][:],
                                             start=True, stop=True,
                                             skip_group_check=True)
                        pt = stream.tile([128, 1024], BF16, tag="pt", bufs=8)
                        nc.scalar.activation(pt[:], ps_s[:], AF.Exp, scale=SCALE)
                        if kt % 2 == 0:
                            pt_prev = pt
                        else:
                            pair = ev.tile([128, 1024], BF16, tag="pair",
                                           bufs=2, name="pair")
                            nc.vector.tensor_add(pair[:], pt_prev[:], pt[:])
                            if kt == 1:
                                nc.vector.tensor_copy(acc[:], pair[:])
                            else:
                                nc.vector.tensor_add(acc[:], acc[:], pair[:])
                        pv_pend.append((kt, pt))
                        if len(pv_pend) > 2:
                            emit_pv(*pv_pend.pop(0))
                        if finish is not None:
                            if kt == 1:
                                finish(0)
                            elif kt == 2:
                                finish(1)
                            elif kt == 4:
                                finish(2)
                                finish = None
                        if qb > 0 and kt in OP_KTS:
                            outproj_mo(qb - 1, h * 8 + OP_KTS[kt],
                                       oth_tag, "v")
                    finish = make_finish(emit_pv, list(pv_pend), acc, ps_o,
                                         h, qsl)

            for phase in range(3):
                finish(phase)
            # tail: last q block's out-projection, double-buffered across
            # the pC/pD tags with evictions split over vector+scalar
            for mo in range(D // 128):
                outproj_mo(NQB - 1, mo, "pC" if mo % 2 else "pD",
                           "s" if mo % 2 else "v")

    nc.compile()
    return nc


_NC_CACHE = None


def _get_nc():
    global _NC_CACHE
    if _NC_CACHE is None:
        _NC_CACHE = build()
    return _NC_CACHE


def _ensure_axon_hooks_stub():
    """bass_utils imports antenv.axon_hooks when tracing is requested via env;
    provide a no-op stub if the image lacks it so a stray BASS_TRACE cannot
    crash the run."""
    import types
    try:
        from antenv import axon_hooks  # noqa: F401
        return
    except Exception:
        pass
    try:
        import antenv
        m = types.ModuleType("antenv.axon_hooks")
        m.set_axon_ntff_profile_hook = lambda h: None
        m.get_axon_ntff_profile_hook = lambda: None
        sys.modules["antenv.axon_hooks"] = m
        antenv.axon_hooks = m
    except Exception:
        pass


def kernel(x, wq, wk, wv, wo, q_norm_w, k_norm_w):
    import ml_dtypes
    from concourse import bass_utils

    _ensure_axon_hooks_stub()

    x = np.asarray(x, dtype=np.float32)
    wq = np.asarray(wq, dtype=np.float32)
    wk = np.asarray(wk, dtype=np.float32)
    wv = np.asarray(wv, dtype=np.float32)
    wo = np.asarray(wo, dtype=np.float32)
    q_norm_w = np.asarray(q_norm_w, dtype=np.float32).reshape(DH, 1)
    k_norm_w = np.asarray(k_norm_w, dtype=np.float32).reshape(DH, 1)

    B = x.shape[0]
    xTb = np.ascontiguousarray(x.reshape(S, D).T).astype(ml_dtypes.bfloat16)

    in_maps = []
    for c in range(NC):
        hsl = slice(c * DHC, (c + 1) * DHC)
        in_maps.append({
            "xTb": xTb,
            "wqb": np.ascontiguousarray(wq[hsl, :].T).astype(ml_dtypes.bfloat16),
            "wkb": np.ascontiguousarray(wk[hsl, :].T).astype(ml_dtypes.bfloat16),
            "wvb": np.ascontiguousarray(wv[hsl, :].T).astype(ml_dtypes.bfloat16),
            "wob": np.ascontiguousarray(wo[:, hsl].T).astype(ml_dtypes.bfloat16),
            "qw": q_norm_w,
            "kw": k_norm_w,
            "ones_c": np.ones((128, 1), dtype=ml_dtypes.bfloat16),
        })

    nc = _get_nc()
    res = bass_utils.run_bass_kernel_spmd(
        nc, in_maps, core_ids=list(range(NC)), trace=TRACE,
    )
    acc = res.results[0]["outT"]
    for c in range(1, NC):
        acc = acc + res.results[c]["outT"]
    out = np.ascontiguousarray(acc.T).reshape(B, S, D)
    if TRACE:
        kernel.last_exec_time_ns = res.exec_time_ns
        kernel.last_results = res
    return out


# revision 20
# speedup vs baseline: 1.1163x; 1.0086x over previous
"""Multi-head self-attention (B=1, S=4096, D=2048, H=16, Dh=128) on 8 TRN2
NeuronCores. Head-sharded tensor parallelism: each core computes 2 heads end to
end in transposed layout, writes its partial out-projection [D, S]; the host
sums the 8 partials and transposes back to [S, D].

Dtype strategy: activations/weights stream as bf16 (matmul inputs), all matmul
accumulation is fp32 in PSUM; softmax statistics accumulate in bf16 on the DVE
(2x mode) and are partition-reduced on the GpSimd/Pool engine so the Activation
engine runs exp back-to-back.  Attention scores are computed in S^T layout
[k, q] so the softmax reduction needs no transposes anywhere.

Scheduling notes:
- One flat pool scope: no mid-kernel pool-drain barriers.  PSUM runs on four
  shared [128,1024] tags (pA/pB = scores double-buffer, pC/pD = PV accumulator
  + out-proj staging, alternating per (qb,h)).
- q/k/v activations live in PER-CHUNK tiles so phase-2 reads depend only on
  the producing chunk's eviction, not on the whole phase-1 sweep.
- The (qb,h) normalize chain is long (gpsimd all-reduce ~6.7us); its reciprocal
  + PV-scale are DEFERRED into the next block's kt loop (kt=6) so they never
  head-of-line-block the in-order DVE queue, and the next block's interleaved
  out-proj starts at kt=11, after the previous normalize has drained.
- Engine budget per core (phase 2): Scalar=exp only (~1.13us/kt), PE=scores+
  PV+out-proj (~1.07us/kt), DVE=bf16 adds+evictions, Pool=partition reductions.
"""
import sys
import numpy as np

for _p in ("/opt/trn_rl_repo",):
    if _p not in sys.path:
        sys.path.append(_p)

import concourse.bacc as bacc
import concourse.mybir as mybir
import concourse.tile as tile
from concourse import bass_isa

F32 = mybir.dt.float32
F32R = mybir.dt.float32r
BF16 = mybir.dt.bfloat16
AF = mybir.ActivationFunctionType
MUL = mybir.AluOpType.mult
RADD = bass_isa.ReduceOp.add

D = 2048            # d_model
S = 4096            # sequence length
DH = 128            # head dim
HPC = 2             # heads per core
DHC = HPC * DH      # 256 head-dims per core
NC = 8              # cores
EPS = 1e-6
SCALE = 1.0 / np.sqrt(DH)

NCH = S // 512      # 8 seq chunks of 512
KT_D = D // 128     # 16 k-tiles over d_model
KT_S = S // 128     # 32 k-tiles over sequence

# kt slots (within the following (qb,h) block) at which the previous q-block's
# out-projection tiles are emitted: late enough that the previous block's
# normalize chain has drained, spread every 3rd kt to balance the PE burst
# against the exp cadence.
OP_KTS = {9 + 3 * j: j for j in range(8)}

TRACE = False       # set by test harness for profiling runs


def build():
    nc = bacc.Bacc("TRN2", target_bir_lowering=False, debug=False)

    xTb = nc.dram_tensor("xTb", [D, S], BF16, kind="ExternalInput")
    wqb = nc.dram_tensor("wqb", [D, DHC], BF16, kind="ExternalInput")
    wkb = nc.dram_tensor("wkb", [D, DHC], BF16, kind="ExternalInput")
    wvb = nc.dram_tensor("wvb", [D, DHC], BF16, kind="ExternalInput")
    wob = nc.dram_tensor("wob", [DHC, D], BF16, kind="ExternalInput")
    qw = nc.dram_tensor("qw", [DH, 1], F32, kind="ExternalInput")
    kw = nc.dram_tensor("kw", [DH, 1], F32, kind="ExternalInput")
    ones_c_d = nc.dram_tensor("ones_c", [128, 1], BF16, kind="ExternalInput")
    outT = nc.dram_tensor("outT", [D, S], F32, kind="ExternalOutput")

    # batched-DMA views: whole weight matrices / x chunks in one transfer
    xTb_c = xTb.rearrange("(kt p) s -> p kt s", p=128)      # [128,16,4096]
    wq_a = wqb.rearrange("(kt p) m -> p kt m", p=128)       # [128,16,256]
    wk_a = wkb.rearrange("(kt p) m -> p kt m", p=128)
    wv_a = wvb.rearrange("(kt p) m -> p kt m", p=128)
    wo_a = wob.rearrange("(kt p) m -> p kt m", p=128)       # [128,2,2048]
    outT_t = outT.rearrange("(mo p) s -> mo p s", p=128)    # [16,128,4096]

    with tile.TileContext(nc) as tc, \
         nc.allow_low_precision(reason="bf16 compute is intentional"):
        with (
            tc.tile_pool(name="consts", bufs=1) as consts,
            tc.tile_pool(name="big", bufs=1) as big,
            tc.tile_pool(name="stream", bufs=6) as stream,
            tc.tile_pool(name="ev", bufs=1) as ev,
            tc.tile_pool(name="ps", bufs=1, space="PSUM") as ps,
        ):
            # ---- residents ----
            ones_col = consts.tile([128, 1], BF16)         # lhsT for rms sums
            eps_sb = consts.tile([1, 1], F32, tag="eps")
            qw_sb = consts.tile([DH, 1], F32, tag="qw")    # per-partition norm w
            kw_sb = consts.tile([DH, 1], F32, tag="kw")

            # per-chunk activation tiles: fine-grained phase-2 dependencies
            qTc = [[big.tile([128, 512], BF16, tag=f"q{h}c{c}", name=f"qT{h}_{c}")
                    for c in range(NCH)] for h in range(HPC)]
            kTc = [[big.tile([128, 512], BF16, tag=f"k{h}c{c}", name=f"kT{h}_{c}")
                    for c in range(NCH)] for h in range(HPC)]
            vc = [big.tile([128, 4, DHC], BF16, tag=f"vc{c}", name=f"v_{c}")
                  for c in range(NCH)]
            o_sb = [big.tile([128, S], BF16, tag=f"o{h}", name=f"o{h}")
                    for h in range(HPC)]
            wo_sb = big.tile([128, HPC, D], BF16, tag="wo")
            wq_sb = big.tile([128, KT_D, DHC], BF16, tag="wq")
            wk_sb = big.tile([128, KT_D, DHC], BF16, tag="wk")
            wv_sb = big.tile([128, KT_D, DHC], BF16, tag="wv")

            # ========== Phase 1: q/k/v projections + q/k rmsnorm ==========
            # Single pass over x^T: per (chunk, kt) one x tile feeds 2 q-mms,
            # 2 k-mms and 4 v-mms.  The 4 v accumulators pack two [128,256]
            # groups per PSUM bank.  DMAs execute serially on the queue, so
            # issue order is wq, x0, wk, wv (first matmul needs wq+x0 only).
            nc.sync.dma_start(out=wq_sb[:], in_=wq_a)
            xc = [None] * NCH
            xc[0] = stream.tile([128, KT_D, 512], BF16, tag="xc", bufs=2,
                                name="xc0")
            nc.sync.dma_start(out=xc[0][:], in_=xTb_c[:, :, 0:512])
            nc.sync.dma_start(out=wk_sb[:], in_=wk_a)
            nc.sync.dma_start(out=wv_sb[:], in_=wv_a)
            nc.sync.dma_start(out=qw_sb[:], in_=qw[:])
            nc.sync.dma_start(out=kw_sb[:], in_=kw[:])
            nc.sync.dma_start(out=ones_col[:], in_=ones_c_d[:])
            nc.vector.memset(eps_sb[:], EPS)
            nc.sync.dma_start(out=wo_sb[:], in_=wo_a)

            def make_stats(n, ps_k, ps_q, vv):
                """Deferred per-chunk eviction block: v first (phase 2's PV
                needs it soonest for chunk 7), then q stats (gates the next
                chunk's q-pass via pB), then k.  The ss row sums reuse chunk
                n's OWN k-accumulator tag — its readers are emitted in this
                very block, so the rotation never clobbers a live tile."""
                ss_tag = "pA" if n % 2 == 0 else "pD"
                def run():
                    for i in range(2):
                        nc.scalar.copy(
                            vc[n][:, 2 * i:2 * i + 2, :].rearrange(
                                "p a b -> p (a b)"),
                            vv[:, i * 512:(i + 1) * 512])
                    sqs, raws = {}, {}
                    for key, ps_list in (("q", ps_q), ("k", ps_k)):
                        for m in range(HPC):
                            sq = ev.tile([128, 512], BF16, tag="sq", bufs=4,
                                         name="sq")
                            nc.scalar.activation(sq[:], ps_list[m], AF.Square)
                            raw = ev.tile([128, 512], F32, tag="raw", bufs=4,
                                          name="raw")
                            nc.vector.tensor_copy(raw[:], ps_list[m])
                            sqs[key, m] = sq
                            raws[key, m] = raw
                    for key, dst, w_col in (("q", qTc, qw_sb), ("k", kTc, kw_sb)):
                        for m in range(HPC):
                            ps_ss = ps.tile([1, 512], F32, tag=ss_tag,
                                            name="ps_ss")
                            nc.tensor.matmul(ps_ss[:], ones_col[:],
                                             sqs[key, m][:],
                                             start=True, stop=True,
                                             skip_group_check=True)
                            ms_row = ev.tile([1, 512], F32, tag="msr", bufs=1)
                            nc.scalar.activation(ms_row[:], ps_ss[:],
                                                 AF.Identity,
                                                 bias=eps_sb[:],
                                                 scale=1.0 / 128.0)
                            rec = ev.tile([1, 512], F32, tag="rec", bufs=1)
                            nc.vector.reciprocal_approx_fast(out=rec[:],
                                                             in_=ms_row[:])
                            rrms = ev.tile([1, 512], F32R, tag="rrms", bufs=1)
                            nc.scalar.activation(rrms[:], rec[:], AF.Sqrt)
                            rb = ev.tile([128, 512], F32R, tag="rb", bufs=2)
                            nc.gpsimd.partition_broadcast(rb[:], rrms[:])
                            nc.vector.scalar_tensor_tensor(
                                dst[m][n][:], raws[key, m][:], w_col[:], rb[:],
                                op0=MUL, op1=MUL)
                return run

            stats = None
            for n in range(NCH):
                if n + 1 < NCH:
                    xc[n + 1] = stream.tile([128, KT_D, 512], BF16,
                                            tag="xc", bufs=2,
                                            name=f"xc{n+1}")
                    nc.sync.dma_start(
                        out=xc[n + 1][:],
                        in_=xTb_c[:, :, (n + 1) * 512:(n + 2) * 512])
                # k accumulator alternates pA/pD per chunk: the next chunk's
                # k-pass starts with no WAR wait on this chunk's evictions
                kk = ps.tile([128, 1024], F32, tag=("pA" if n % 2 == 0 else "pD"),
                             name="kk")
                ps_k = [kk[:, m * 512:(m + 1) * 512] for m in range(HPC)]
                # three passes over the resident x chunk: k, then (after the
                # previous chunk's deferred stats) q, then v
                for kt in range(KT_D):
                    for m in range(HPC):
                        nc.tensor.matmul(ps_k[m],
                                         wk_sb[:, kt, m * DH:(m + 1) * DH],
                                         xc[n][:, kt, :],
                                         start=(kt == 0), stop=(kt == KT_D - 1),
                                         skip_group_check=True)
                if stats is not None:
                    stats()
                qq = ps.tile([128, 1024], F32, tag="pB", name="qq")
                vv = ps.tile([128, 1024], F32, tag="pC", name="vv")
                ps_q = [qq[:, m * 512:(m + 1) * 512] for m in range(HPC)]
                for kt in range(KT_D):
                    for m in range(HPC):
                        nc.tensor.matmul(ps_q[m],
                                         wq_sb[:, kt, m * DH:(m + 1) * DH],
                                         xc[n][:, kt, :],
                                         start=(kt == 0), stop=(kt == KT_D - 1),
                                         skip_group_check=True)
                for kt in range(KT_D):
                    for sm in range(4):
                        pv = vv[:, sm * 256:(sm + 1) * 256]
                        nc.tensor.matmul(pv,
                                         xc[n][:, kt, sm * 128:(sm + 1) * 128],
                                         wv_sb[:, kt, :],
                                         start=(kt == 0 and sm % 2 == 0),
                                         stop=(kt == KT_D - 1),
                                         skip_group_check=True)
                stats = make_stats(n, ps_k, ps_q, vv)
            stats()

            # ============ Phase 2+3: attention + out-projection ============
            # 1024-wide q blocks; S^T scores span two PSUM banks. Scalar does
            # ONLY exp; softmax sums accumulate in bf16 on DVE (2x mode) and
            # partition-reduce on GpSimd.
            NQB = S // 1024

            def outproj_mo(qb, mo, ytag, yeng):
                mosl = slice(mo * 128, (mo + 1) * 128)
                ps_y = ps.tile([128, 1024], F32, tag=ytag, name="ps_y")
                for h2 in range(HPC):
                    for u in range(2):
                        nc.tensor.matmul(ps_y[:, u * 512:(u + 1) * 512],
                                         wo_sb[:, h2, mosl],
                                         o_sb[h2][:, qb * 1024 + u * 512:
                                                   qb * 1024 + (u + 1) * 512],
                                         start=(h2 == 0), stop=(h2 == HPC - 1),
                                         skip_group_check=True)
                y = stream.tile([128, 1024], F32, tag="y", bufs=4)
                if yeng == "vs":
                    nc.vector.tensor_copy(y[:, :512], ps_y[:, :512])
                    nc.scalar.copy(y[:, 512:], ps_y[:, 512:])
                elif yeng == "s":
                    nc.scalar.copy(y[:], ps_y[:])
                else:
                    nc.vector.tensor_copy(y[:], ps_y[:])
                nc.sync.dma_start(out=outT_t[mo][:, qb * 1024:(qb + 1) * 1024],
                                  in_=y[:])

            # Deferred finish of a (qb,h) block, interleaved into the next
            # block's early kt slots so the in-order engines never stall on
            # the cross-engine normalize chain.
            def make_finish(emit_pv_fn, pv_rest, acc_c, ps_o_c, h_c, qsl_c):
                st = {}

                def step(phase):
                    if phase == 0:          # leftover PV accumulations
                        for kt2, pt2 in pv_rest:
                            emit_pv_fn(kt2, pt2)
                    elif phase == 1:        # partition sum + reciprocal
                        ps_se = ps.tile([1, 1024], F32, tag="pB", name="ps_se")
                        for u in range(2):
                            nc.tensor.matmul(ps_se[:, u * 512:(u + 1) * 512],
                                             ones_col[:],
                                             acc_c[:, u * 512:(u + 1) * 512],
                                             start=True, stop=True,
                                             skip_group_check=True)
                        se_row = ev.tile([1, 1024], F32, tag="ser", bufs=1)
                        nc.vector.tensor_copy(se_row[:], ps_se[:])
                        rec2 = ev.tile([1, 1024], F32, tag="rec2", bufs=1)
                        nc.vector.reciprocal_approx_fast(out=rec2[:],
                                                         in_=se_row[:])
                        st["rec2"] = rec2
                    else:                   # broadcast + PV scale
                        rb2 = ev.tile([128, 1024], F32, tag="rb2", bufs=2)
                        nc.gpsimd.partition_broadcast(rb2[:], st["rec2"][:])
                        nc.vector.tensor_mul(o_sb[h_c][:, qsl_c],
                                             ps_o_c[:], rb2[:])
                return step

            finish = None
            for qb in range(NQB):
                qsl = slice(qb * 1024, (qb + 1) * 1024)
                for h in range(HPC):
                    idx = qb * HPC + h
                    pso_tag = "pC" if idx % 2 == 0 else "pD"
                    oth_tag = "pD" if idx % 2 == 0 else "pC"
                    ps_o = ps.tile([128, 1024], F32, tag=pso_tag,
                                   name=f"ps_o{idx % 2}")
                    acc = ev.tile([128, 1024], BF16, tag="acc", bufs=2,
                                  name="acc")
                    pt_prev = None

                    def emit_pv(kt2, pt2, ps_o=ps_o, h=h):
                        for u in range(2):
                            nc.tensor.matmul(ps_o[:, u * 512:(u + 1) * 512],
                                             vc[kt2 // 4][:, kt2 % 4,
                                                          h * DH:(h + 1) * DH],
                                             pt2[:, u * 512:(u + 1) * 512],
                                             start=(kt2 == 0),
                                             stop=(kt2 == KT_S - 1),
                                             skip_group_check=True)

                    pv_pend = []
                    for kt in range(KT_S):
                        kq = (kt % 4) * 128
                        ps_s = ps.tile([128, 1024], F32,
                                       tag=("pA" if kt % 2 == 0 else "pB"),
                                       name="ps_s")
                        for u in range(2):
                            nc.tensor.matmul(ps_s[:, u * 512:(u + 1) * 512],
                                             kTc[h][kt // 4][:, kq:kq + 128],
                                             qTc[h][qb * 2 + u][:],
                                             start=True, stop=True,
                                             skip_group_check=True)
                        pt = stream.tile([128, 1024], BF16, tag="pt", bufs=8)
                        nc.scalar.activation(pt[:], ps_s[:], AF.Exp, scale=SCALE)
                        if kt % 2 == 0:
                            pt_prev = pt
                        else:
                            pair = ev.tile([128, 1024], BF16, tag="pair",
                                           bufs=2, name="pair")
                            nc.vector.tensor_add(pair[:], pt_prev[:], pt[:])
                            if kt == 1:
                                nc.vector.tensor_copy(acc[:], pair[:])
                            else:
                                nc.vector.tensor_add(acc[:], acc[:], pair[:])
                        pv_pend.append((kt, pt))
                        if len(pv_pend) > 2:
                            emit_pv(*pv_pend.pop(0))
                        if finish is not None:
                            if kt == 1:
                                finish(0)
                            elif kt == 2:
                                finish(1)
                            elif kt == 4:
                                finish(2)
                                finish = None
                        if qb > 0 and kt in OP_KTS:
                            outproj_mo(qb - 1, h * 8 + OP_KTS[kt],
                                       oth_tag, "v")
                    finish = make_finish(emit_pv, list(pv_pend), acc, ps_o,
                                         h, qsl)

            for phase in range(3):
                finish(phase)
            # tail: last q block's out-projection, double-buffered across
            # the pC/pD tags with evictions split over vector+scalar
            for mo in range(D // 128):
                outproj_mo(NQB - 1, mo, "pC" if mo % 2 else "pD", "vs")

    nc.compile()
    return nc


_NC_CACHE = None


def _get_nc():
    global _NC_CACHE
    if _NC_CACHE is None:
        _NC_CACHE = build()
    return _NC_CACHE


def _ensure_axon_hooks_stub():
    """bass_utils imports antenv.axon_hooks when tracing is requested via env;
    provide a no-op stub if the image lacks it so a stray BASS_TRACE cannot
    crash the run."""
    import types
    try:
        from antenv import axon_hooks  # noqa: F401
        return
    except Exception:
        pass
    try:
        import antenv
        m = types.ModuleType("antenv.axon_hooks")
        m.set_axon_ntff_profile_hook = lambda h: None
        m.get_axon_ntff_profile_hook = lambda: None
        sys.modules["antenv.axon_hooks"] = m
        antenv.axon_hooks = m
    except Exception:
        pass


def kernel(x, wq, wk, wv, wo, q_norm_w, k_norm_w):
    import ml_dtypes
    from concourse import bass_utils

    _ensure_axon_hooks_stub()

    x = np.asarray(x, dtype=np.float32)
    wq = np.asarray(wq, dtype=np.float32)
    wk = np.asarray(wk, dtype=np.float32)
    wv = np.asarray(wv, dtype=np.float32)
    wo = np.asarray(wo, dtype=np.float32)
    q_norm_w = np.asarray(q_norm_w, dtype=np.float32).reshape(DH, 1)
    k_norm_w = np.asarray(k_norm_w, dtype=np.float32).reshape(DH, 1)

    B = x.shape[0]
    xTb = np.ascontiguousarray(x.reshape(S, D).T).astype(ml_dtypes.bfloat16)

    in_maps = []
    for c in range(NC):
        hsl = slice(c * DHC, (c + 1) * DHC)
        in_maps.append({
            "xTb": xTb,
            "wqb": np.ascontiguousarray(wq[hsl, :].T).astype(ml_dtypes.bfloat16),
            "wkb": np.ascontiguousarray(wk[hsl, :].T).astype(ml_dtypes.bfloat16),
            "wvb": np.ascontiguousarray(wv[hsl, :].T).astype(ml_dtypes.bfloat16),
            "wob": np.ascontiguousarray(wo[:, hsl].T).astype(ml_dtypes.bfloat16),
            "qw": q_norm_w,
            "kw": k_norm_w,
            "ones_c": np.ones((128, 1), dtype=ml_dtypes.bfloat16),
        })

    nc = _get_nc()
    res = bass_utils.run_bass_kernel_spmd(
        nc, in_maps, core_ids=list(range(NC)), trace=TRACE,
    )
    acc = res.results[0]["outT"]
    for c in range(1, NC):
        acc = acc + res.results[c]["outT"]
    out = np.ascontiguousarray(acc.T).reshape(B, S, D)
    if TRACE:
        kernel.last_exec_time_ns = res.exec_time_ns
        kernel.last_results = res
    return out


# revision 23
# speedup vs baseline: 1.1226x; 1.0056x over previous
"""Multi-head self-attention (B=1, S=4096, D=2048, H=16, Dh=128) on 8 TRN2
NeuronCores. Head-sharded tensor parallelism: each core computes 2 heads end to
end in transposed layout, writes its partial out-projection [D, S]; the host
sums the 8 partials and transposes back to [S, D].

Dtype strategy: activations/weights stream as bf16 (matmul inputs), all matmul
accumulation is fp32 in PSUM; softmax statistics accumulate in bf16 on the DVE
(2x mode) and are partition-reduced on the GpSimd/Pool engine so the Activation
engine runs exp back-to-back.  Attention scores are computed in S^T layout
[k, q] so the softmax reduction needs no transposes anywhere.

Scheduling notes:
- One flat pool scope: no mid-kernel pool-drain barriers.  PSUM runs on four
  shared [128,1024] tags (pA/pB = scores double-buffer, pC/pD = PV accumulator
  + out-proj staging, alternating per (qb,h)).
- q/k/v activations live in PER-CHUNK tiles so phase-2 reads depend only on
  the producing chunk's eviction, not on the whole phase-1 sweep.
- The (qb,h) normalize chain is long (gpsimd all-reduce ~6.7us); its reciprocal
  + PV-scale are DEFERRED into the next block's kt loop (kt=6) so they never
  head-of-line-block the in-order DVE queue, and the next block's interleaved
  out-proj starts at kt=11, after the previous normalize has drained.
- Engine budget per core (phase 2): Scalar=exp only (~1.13us/kt), PE=scores+
  PV+out-proj (~1.07us/kt), DVE=bf16 adds+evictions, Pool=partition reductions.
"""
import sys
import numpy as np

for _p in ("/opt/trn_rl_repo",):
    if _p not in sys.path:
        sys.path.append(_p)

import concourse.bacc as bacc
import concourse.mybir as mybir
import concourse.tile as tile
from concourse import bass_isa

F32 = mybir.dt.float32
F32R = mybir.dt.float32r
BF16 = mybir.dt.bfloat16
AF = mybir.ActivationFunctionType
MUL = mybir.AluOpType.mult
RADD = bass_isa.ReduceOp.add

D = 2048            # d_model
S = 4096            # sequence length
DH = 128            # head dim
HPC = 2             # heads per core
DHC = HPC * DH      # 256 head-dims per core
NC = 8              # cores
EPS = 1e-6
SCALE = 1.0 / np.sqrt(DH)

NCH = S // 512      # 8 seq chunks of 512
KT_D = D // 128     # 16 k-tiles over d_model
KT_S = S // 128     # 32 k-tiles over sequence

# kt slots (within the following (qb,h) block) at which the previous q-block's
# out-projection tiles are emitted: late enough that the previous block's
# normalize chain has drained, spread every 3rd kt to balance the PE burst
# against the exp cadence.
OP_KTS = {9 + 3 * j: j for j in range(8)}

TRACE = False       # set by test harness for profiling runs


def build():
    nc = bacc.Bacc("TRN2", target_bir_lowering=False, debug=False)

    xTb = nc.dram_tensor("xTb", [D, S], BF16, kind="ExternalInput")
    wqb = nc.dram_tensor("wqb", [D, DHC], BF16, kind="ExternalInput")
    wkb = nc.dram_tensor("wkb", [D, DHC], BF16, kind="ExternalInput")
    wvb = nc.dram_tensor("wvb", [D, DHC], BF16, kind="ExternalInput")
    wob = nc.dram_tensor("wob", [DHC, D], BF16, kind="ExternalInput")
    qw = nc.dram_tensor("qw", [DH, 1], F32, kind="ExternalInput")
    kw = nc.dram_tensor("kw", [DH, 1], F32, kind="ExternalInput")
    ones_c_d = nc.dram_tensor("ones_c", [128, 1], BF16, kind="ExternalInput")
    outT = nc.dram_tensor("outT", [D, S], F32, kind="ExternalOutput")

    # batched-DMA views: whole weight matrices / x chunks in one transfer
    xTb_c = xTb.rearrange("(kt p) s -> p kt s", p=128)      # [128,16,4096]
    wq_a = wqb.rearrange("(kt p) m -> p kt m", p=128)       # [128,16,256]
    wk_a = wkb.rearrange("(kt p) m -> p kt m", p=128)
    wv_a = wvb.rearrange("(kt p) m -> p kt m", p=128)
    wo_a = wob.rearrange("(kt p) m -> p kt m", p=128)       # [128,2,2048]
    outT_t = outT.rearrange("(mo p) s -> mo p s", p=128)    # [16,128,4096]

    with tile.TileContext(nc) as tc, \
         nc.allow_low_precision(reason="bf16 compute is intentional"):
        with (
            tc.tile_pool(name="consts", bufs=1) as consts,
            tc.tile_pool(name="big", bufs=1) as big,
            tc.tile_pool(name="stream", bufs=6) as stream,
            tc.tile_pool(name="ev", bufs=1) as ev,
            tc.tile_pool(name="ps", bufs=1, space="PSUM") as ps,
        ):
            # ---- residents ----
            ones_col = consts.tile([128, 1], BF16)         # lhsT for rms sums
            eps_sb = consts.tile([1, 1], F32, tag="eps")
            qw_sb = consts.tile([DH, 1], F32, tag="qw")    # per-partition norm w
            kw_sb = consts.tile([DH, 1], F32, tag="kw")

            # per-chunk activation tiles: fine-grained phase-2 dependencies
            qTc = [[big.tile([128, 512], BF16, tag=f"q{h}c{c}", name=f"qT{h}_{c}")
                    for c in range(NCH)] for h in range(HPC)]
            kTc = [[big.tile([128, 512], BF16, tag=f"k{h}c{c}", name=f"kT{h}_{c}")
                    for c in range(NCH)] for h in range(HPC)]
            vc = [big.tile([128, 4, DHC], BF16, tag=f"vc{c}", name=f"v_{c}")
                  for c in range(NCH)]
            o_sb = [big.tile([128, S], BF16, tag=f"o{h}", name=f"o{h}")
                    for h in range(HPC)]
            wo_sb = big.tile([128, HPC, D], BF16, tag="wo")
            wq_sb = big.tile([128, KT_D, DHC], BF16, tag="wq")
            wk_sb = big.tile([128, KT_D, DHC], BF16, tag="wk")
            wv_sb = big.tile([128, KT_D, DHC], BF16, tag="wv")

            # ========== Phase 1: q/k/v projections + q/k rmsnorm ==========
            # Per chunk, three passes over a resident x chunk (k, q, v) into
            # the shared PSUM tags.  DMAs execute serially on the queue and
            # the k-pass runs first, so issue order is wk, x0, wq, wv.
            nc.sync.dma_start(out=wk_sb[:], in_=wk_a)
            xc = [None] * NCH
            xc[0] = stream.tile([128, KT_D, 512], BF16, tag="xc", bufs=2,
                                name="xc0")
            nc.sync.dma_start(out=xc[0][:], in_=xTb_c[:, :, 0:512])
            nc.sync.dma_start(out=wq_sb[:], in_=wq_a)
            nc.sync.dma_start(out=wv_sb[:], in_=wv_a)
            nc.sync.dma_start(out=qw_sb[:], in_=qw[:])
            nc.sync.dma_start(out=kw_sb[:], in_=kw[:])
            nc.sync.dma_start(out=ones_col[:], in_=ones_c_d[:])
            nc.vector.memset(eps_sb[:], EPS)
            nc.sync.dma_start(out=wo_sb[:], in_=wo_a)

            def make_stats(n, ps_k, ps_q, vv):
                """Deferred per-chunk eviction block: v first (phase 2's PV
                needs it soonest for chunk 7), then q stats (gates the next
                chunk's q-pass via pB), then k.  The ss row sums reuse chunk
                n's OWN k-accumulator tag — its readers are emitted in this
                very block, so the rotation never clobbers a live tile."""
                ss_tag = "pA" if n % 2 == 0 else "pD"
                # For the last chunk, keep the scalar engine clear (it must
                # start phase-2 exps immediately): Square and the ms row move
                # to the DVE; only the Sqrt stays on scalar.
                dve_heavy = n == NCH - 1

                def run():
                    for i in range(2):
                        nc.scalar.copy(
                            vc[n][:, 2 * i:2 * i + 2, :].rearrange(
                                "p a b -> p (a b)"),
                            vv[:, i * 512:(i + 1) * 512])
                    sqs, raws = {}, {}
                    for key, ps_list in (("q", ps_q), ("k", ps_k)):
                        for m in range(HPC):
                            raw = ev.tile([128, 512], F32, tag="raw", bufs=4,
                                          name="raw")
                            nc.vector.tensor_copy(raw[:], ps_list[m])
                            sq = ev.tile([128, 512], BF16, tag="sq", bufs=4,
                                         name="sq")
                            if dve_heavy:
                                nc.vector.tensor_mul(sq[:], raw[:], raw[:])
                            else:
                                nc.scalar.activation(sq[:], ps_list[m],
                                                     AF.Square)
                            sqs[key, m] = sq
                            raws[key, m] = raw
                    for key, dst, w_col in (("q", qTc, qw_sb), ("k", kTc, kw_sb)):
                        for m in range(HPC):
                            ps_ss = ps.tile([1, 512], F32, tag=ss_tag,
                                            name="ps_ss")
                            nc.tensor.matmul(ps_ss[:], ones_col[:],
                                             sqs[key, m][:],
                                             start=True, stop=True,
                                             skip_group_check=True)
                            ms_row = ev.tile([1, 512], F32, tag="msr", bufs=1)
                            if dve_heavy:
                                nc.vector.tensor_scalar(
                                    ms_row[:], ps_ss[:], 1.0 / 128.0, EPS,
                                    op0=MUL, op1=mybir.AluOpType.add)
                            else:
                                nc.scalar.activation(ms_row[:], ps_ss[:],
                                                     AF.Identity,
                                                     bias=eps_sb[:],
                                                     scale=1.0 / 128.0)
                            rec = ev.tile([1, 512], F32, tag="rec", bufs=1)
                            nc.vector.reciprocal_approx_fast(out=rec[:],
                                                             in_=ms_row[:])
                            rrms = ev.tile([1, 512], F32R, tag="rrms", bufs=1)
                            nc.scalar.activation(rrms[:], rec[:], AF.Sqrt)
                            rb = ev.tile([128, 512], F32R, tag="rb", bufs=2)
                            nc.gpsimd.partition_broadcast(rb[:], rrms[:])
                            nc.vector.scalar_tensor_tensor(
                                dst[m][n][:], raws[key, m][:], w_col[:], rb[:],
                                op0=MUL, op1=MUL)
                return run

            stats = None
            for n in range(NCH):
                if n + 1 < NCH:
                    xc[n + 1] = stream.tile([128, KT_D, 512], BF16,
                                            tag="xc", bufs=2,
                                            name=f"xc{n+1}")
                    nc.sync.dma_start(
                        out=xc[n + 1][:],
                        in_=xTb_c[:, :, (n + 1) * 512:(n + 2) * 512])
                # k accumulator alternates pA/pD per chunk: the next chunk's
                # k-pass starts with no WAR wait on this chunk's evictions
                kk = ps.tile([128, 1024], F32, tag=("pA" if n % 2 == 0 else "pD"),
                             name="kk")
                ps_k = [kk[:, m * 512:(m + 1) * 512] for m in range(HPC)]
                # three passes over the resident x chunk: k, then (after the
                # previous chunk's deferred stats) q, then v
                for kt in range(KT_D):
                    for m in range(HPC):
                        nc.tensor.matmul(ps_k[m],
                                         wk_sb[:, kt, m * DH:(m + 1) * DH],
                                         xc[n][:, kt, :],
                                         start=(kt == 0), stop=(kt == KT_D - 1),
                                         skip_group_check=True)
                if stats is not None:
                    stats()
                qq = ps.tile([128, 1024], F32, tag="pB", name="qq")
                vv = ps.tile([128, 1024], F32, tag="pC", name="vv")
                ps_q = [qq[:, m * 512:(m + 1) * 512] for m in range(HPC)]
                for kt in range(KT_D):
                    for m in range(HPC):
                        nc.tensor.matmul(ps_q[m],
                                         wq_sb[:, kt, m * DH:(m + 1) * DH],
                                         xc[n][:, kt, :],
                                         start=(kt == 0), stop=(kt == KT_D - 1),
                                         skip_group_check=True)
                for kt in range(KT_D):
                    for sm in range(4):
                        pv = vv[:, sm * 256:(sm + 1) * 256]
                        nc.tensor.matmul(pv,
                                         xc[n][:, kt, sm * 128:(sm + 1) * 128],
                                         wv_sb[:, kt, :],
                                         start=(kt == 0 and sm % 2 == 0),
                                         stop=(kt == KT_D - 1),
                                         skip_group_check=True)
                stats = make_stats(n, ps_k, ps_q, vv)
            stats()

            # ============ Phase 2+3: attention + out-projection ============
            # 1024-wide q blocks; S^T scores span two PSUM banks. Scalar does
            # ONLY exp; softmax sums accumulate in bf16 on DVE (2x mode) and
            # partition-reduce on GpSimd.
            NQB = S // 1024

            def outproj_mo(qb, mo, ytag, yeng):
                mosl = slice(mo * 128, (mo + 1) * 128)
                ps_y = ps.tile([128, 1024], F32, tag=ytag, name="ps_y")
                for h2 in range(HPC):
                    for u in range(2):
                        nc.tensor.matmul(ps_y[:, u * 512:(u + 1) * 512],
                                         wo_sb[:, h2, mosl],
                                         o_sb[h2][:, qb * 1024 + u * 512:
                                                   qb * 1024 + (u + 1) * 512],
                                         start=(h2 == 0), stop=(h2 == HPC - 1),
                                         skip_group_check=True)
                y = stream.tile([128, 1024], F32, tag="y", bufs=4)
                if yeng == "vs":
                    nc.vector.tensor_copy(y[:, :512], ps_y[:, :512])
                    nc.scalar.copy(y[:, 512:], ps_y[:, 512:])
                elif yeng == "s":
                    nc.scalar.copy(y[:], ps_y[:])
                else:
                    nc.vector.tensor_copy(y[:], ps_y[:])
                nc.sync.dma_start(out=outT_t[mo][:, qb * 1024:(qb + 1) * 1024],
                                  in_=y[:])

            # Deferred finish of a (qb,h) block, interleaved into the next
            # block's early kt slots so the in-order engines never stall on
            # the cross-engine normalize chain.
            def make_finish(emit_pv_fn, pv_rest, acc_c, ps_o_c, h_c, qsl_c):
                st = {}

                def step(phase):
                    if phase == 0:          # leftover PV accumulations
                        for kt2, pt2 in pv_rest:
                            emit_pv_fn(kt2, pt2)
                    elif phase == 1:        # partition sum + reciprocal
                        ps_se = ps.tile([1, 1024], F32, tag="pB", name="ps_se")
                        for u in range(2):
                            nc.tensor.matmul(ps_se[:, u * 512:(u + 1) * 512],
                                             ones_col[:],
                                             acc_c[:, u * 512:(u + 1) * 512],
                                             start=True, stop=True,
                                             skip_group_check=True)
                        se_row = ev.tile([1, 1024], F32, tag="ser", bufs=1)
                        nc.vector.tensor_copy(se_row[:], ps_se[:])
                        rec2 = ev.tile([1, 1024], F32, tag="rec2", bufs=1)
                        nc.vector.reciprocal_approx_fast(out=rec2[:],
                                                         in_=se_row[:])
                        st["rec2"] = rec2
                    else:                   # broadcast + PV scale
                        rb2 = ev.tile([128, 1024], F32, tag="rb2", bufs=2)
                        nc.gpsimd.partition_broadcast(rb2[:], st["rec2"][:])
                        nc.vector.tensor_mul(o_sb[h_c][:, qsl_c],
                                             ps_o_c[:], rb2[:])
                return step

            finish = None
            for qb in range(NQB):
                qsl = slice(qb * 1024, (qb + 1) * 1024)
                for h in range(HPC):
                    idx = qb * HPC + h
                    pso_tag = "pC" if idx % 2 == 0 else "pD"
                    oth_tag = "pD" if idx % 2 == 0 else "pC"
                    ps_o = ps.tile([128, 1024], F32, tag=pso_tag,
                                   name=f"ps_o{idx % 2}")
                    acc = ev.tile([128, 1024], BF16, tag="acc", bufs=2,
                                  name="acc")
                    pt_prev = None

                    def emit_pv(kt2, pt2, ps_o=ps_o, h=h):
                        for u in range(2):
                            nc.tensor.matmul(ps_o[:, u * 512:(u + 1) * 512],
                                             vc[kt2 // 4][:, kt2 % 4,
                                                          h * DH:(h + 1) * DH],
                                             pt2[:, u * 512:(u + 1) * 512],
                                             start=(kt2 == 0),
                                             stop=(kt2 == KT_S - 1),
                                             skip_group_check=True)

                    pv_pend = []
                    for kt in range(KT_S):
                        kq = (kt % 4) * 128
                        ps_s = ps.tile([128, 1024], F32,
                                       tag=("pA" if kt % 2 == 0 else "pB"),
                                       name="ps_s")
                        for u in range(2):
                            nc.tensor.matmul(ps_s[:, u * 512:(u + 1) * 512],
                                             kTc[h][kt // 4][:, kq:kq + 128],
                                             qTc[h][qb * 2 + u][:],
                                             start=True, stop=True,
                                             skip_group_check=True)
                        pt = stream.tile([128, 1024], BF16, tag="pt", bufs=8)
                        nc.scalar.activation(pt[:], ps_s[:], AF.Exp, scale=SCALE)
                        if kt % 2 == 0:
                            pt_prev = pt
                        else:
                            pair = ev.tile([128, 1024], BF16, tag="pair",
                                           bufs=2, name="pair")
                            nc.vector.tensor_add(pair[:], pt_prev[:], pt[:])
                            if kt == 1:
                                nc.vector.tensor_copy(acc[:], pair[:])
                            else:
                                nc.vector.tensor_add(acc[:], acc[:], pair[:])
                        pv_pend.append((kt, pt))
                        if len(pv_pend) > 2:
                            emit_pv(*pv_pend.pop(0))
                        if finish is not None:
                            if kt == 1:
                                finish(0)
                            elif kt == 2:
                                finish(1)
                            elif kt == 4:
                                finish(2)
                                finish = None
                        if qb > 0 and kt in OP_KTS:
                            outproj_mo(qb - 1, h * 8 + OP_KTS[kt],
                                       oth_tag, "v")
                    finish = make_finish(emit_pv, list(pv_pend), acc, ps_o,
                                         h, qsl)

            for phase in range(3):
                finish(phase)
            # tail: last q block's out-projection, double-buffered across
            # the pC/pD tags with evictions split over vector+scalar
            for mo in range(D // 128):
                outproj_mo(NQB - 1, mo, "pC" if mo % 2 else "pD", "vs")

    nc.compile()
    return nc


_NC_CACHE = None


def _get_nc():
    global _NC_CACHE
    if _NC_CACHE is None:
        _NC_CACHE = build()
    return _NC_CACHE


def _ensure_axon_hooks_stub():
    """bass_utils imports antenv.axon_hooks when tracing is requested via env;
    provide a no-op stub if the image lacks it so a stray BASS_TRACE cannot
    crash the run."""
    import types
    try:
        from antenv import axon_hooks  # noqa: F401
        return
    except Exception:
        pass
    try:
        import antenv
        m = types.ModuleType("antenv.axon_hooks")
        m.set_axon_ntff_profile_hook = lambda h: None
        m.get_axon_ntff_profile_hook = lambda: None
        sys.modules["antenv.axon_hooks"] = m
        antenv.axon_hooks = m
    except Exception:
        pass


def kernel(x, wq, wk, wv, wo, q_norm_w, k_norm_w):
    import ml_dtypes
    from concourse import bass_utils

    _ensure_axon_hooks_stub()

    x = np.asarray(x, dtype=np.float32)
    wq = np.asarray(wq, dtype=np.float32)
    wk = np.asarray(wk, dtype=np.float32)
    wv = np.asarray(wv, dtype=np.float32)
    wo = np.asarray(wo, dtype=np.float32)
    q_norm_w = np.asarray(q_norm_w, dtype=np.float32).reshape(DH, 1)
    k_norm_w = np.asarray(k_norm_w, dtype=np.float32).reshape(DH, 1)

    B = x.shape[0]
    xTb = np.ascontiguousarray(x.reshape(S, D).T).astype(ml_dtypes.bfloat16)

    in_maps = []
    for c in range(NC):
        hsl = slice(c * DHC, (c + 1) * DHC)
        in_maps.append({
            "xTb": xTb,
            "wqb": np.ascontiguousarray(wq[hsl, :].T).astype(ml_dtypes.bfloat16),
            "wkb": np.ascontiguousarray(wk[hsl, :].T).astype(ml_dtypes.bfloat16),
            "wvb": np.ascontiguousarray(wv[hsl, :].T).astype(ml_dtypes.bfloat16),
            "wob": np.ascontiguousarray(wo[:, hsl].T).astype(ml_dtypes.bfloat16),
            "qw": q_norm_w,
            "kw": k_norm_w,
            "ones_c": np.ones((128, 1), dtype=ml_dtypes.bfloat16),
        })

    nc = _get_nc()
    res = bass_utils.run_bass_kernel_spmd(
        nc, in_maps, core_ids=list(range(NC)), trace=TRACE,
    )
    acc = res.results[0]["outT"]
    for c in range(1, NC):
        acc = acc + res.results[c]["outT"]
    out = np.ascontiguousarray(acc.T).reshape(B, S, D)
    if TRACE:
        kernel.last_exec_time_ns = res.exec_time_ns
        kernel.last_results = res
    return out
